# revision 46
# baseline (speedup 1.0000x reference)
"""GAT (graph attention network) forward pass on 8 Trainium2 NeuronCores.

Problem: nn_GAT - N=4096 nodes, F=512 features, H=8 heads, 1% dense adjacency.
    heads:  Wh = x @ Ws[h]; e = lrelu(s1[i]+s2[j]); att = masked softmax; elu(att @ Wh)
    out layer: same attention structure on hcat @ Wo, then elu.

Strategy (row-sharded across 8 cores, 3 launches):
  k0: each core computes Wh (all heads, fused matmul) + score vectors for its
      512 nodes; host gathers.
  k1: each core runs 8-head masked-softmax attention for its 512 query rows
      (key insight: exp(lrelu(e)) = max(exp(e), exp(0.2e)) and exp(e) factors
      rank-1 as exp(s1)[i]*exp(s2)[j], so the NxN tiles need NO transcendentals
      and NO PSUM e-matrix - just tensor_scalar/tensor_tensor ops in fp16.
      Softmax is invariant to per-query-row scaling, so scaling row i by
      exp(-s1[i]) turns the exp(e) branch into a per-partition scalar:
        u'[j,i] = exp(s2[j])          (tensor_scalar max)
        v'[j,i] = exp(-0.8*s1[i]) * exp(0.2*s2[j])   (tensor_scalar mult)
        p = max(u', v') * adjT        (mask multiply, fp16)
      The softmax denominator comes free as a ones-column in the value matrix.
      Blocks are routed across DVE, ACT (additive-mask PE+activation form) and
      GPSIMD in proportion to their simulated throughput.
      Also computes hcat @ Wo (+ output-layer score vectors) for its rows.
  k2: output-layer attention for the core's 512 rows; final ELU.

adj is passed from host as a pre-transposed fp16 (exact for a 0/1 mask) slice
per core; x is passed pre-transposed fp32 (pure layout prep, no FLOPs).
"""

import sys

for _p in ("/opt/trn_rl_repo",):
    if _p not in sys.path:
        sys.path.insert(0, _p)

import numpy as np

import concourse.bass as bass
import concourse.tile as tile
from concourse import bacc, mybir
from concourse.bass_utils import run_bass_kernel_spmd
from concourse.masks import make_identity

N, F, H, NH = 4096, 512, 8, 64
M = 8            # cores
R = N // M       # 512 query rows per core
JB = N // 128    # 32 key blocks
IC = R // 128    # 4 query-row chunks per core
HC = NH + 1      # 65 value cols per head (64 + ones col for row sums)
ALPHA = 0.2     # leaky relu slope
BIG = 200.0     # additive mask; 0.2*BIG=40 so masked exp underflows to 0
f32 = mybir.dt.float32
f16 = mybir.dt.float16
OP = mybir.AluOpType
AF = mybir.ActivationFunctionType

_CACHE = {}


def _run(nc, in_maps, core_ids, tries=3):
    """run_bass_kernel_spmd with retry: the axon-tunneled devices
    occasionally report NRT_EXEC_UNIT_UNRECOVERABLE transiently."""
    import time as _time

    for attempt in range(tries):
        try:
            return run_bass_kernel_spmd(nc, in_maps, core_ids=core_ids)
        except Exception:
            if attempt == tries - 1:
                raise
            _time.sleep(5.0)


# ---------------------------------------------------------------- k0
def _build_k0():
    """Per-core: Wh projection for this core's R nodes, all heads fused.

    in:  xT    [F, R] f32 / xT16 [F, R] f16 (this core's x rows, transposed)
         wsa16 [F, F] f16  (Ws stacked, head-major cols)
         wsc   [F, 6H] f32 (score cols: [ws2|.2ws2|-.8ws1|-ws1|ws1|ws2])
    out: wha16 [R, H*HC] f16 (per head: 64 value cols + ones col)
         ev    [R, 6H] f32 (exp(s2)|exp(.2s2)|exp(-.8s1)|exp(-s1)|s1|s2)
    """
    nc = bacc.Bacc("TRN2", target_bir_lowering=False, debug=False, num_devices=M)
    xT = nc.dram_tensor("xT", [F, R], f32, kind="ExternalInput").ap()
    xT16 = nc.dram_tensor("xT16", [F, R], f16, kind="ExternalInput").ap()
    wsa16 = nc.dram_tensor("wsa16", [F, F], f16, kind="ExternalInput").ap()
    wsc = nc.dram_tensor("wsc", [F, 6 * H], f32, kind="ExternalInput").ap()
    wha16 = nc.dram_tensor("wha16", [R, H * HC], f16, kind="ExternalOutput").ap()
    ev = nc.dram_tensor("ev", [R, 6 * H], f32, kind="ExternalOutput").ap()

    with tile.TileContext(nc) as tc:
        with (
            tc.tile_pool(name="sb", bufs=1) as sb,
            tc.tile_pool(name="ps", bufs=4, space="PSUM") as ps,
            tc.tile_pool(name="ob", bufs=4) as ob,
        ):
            # coalesced resident loads (few big DMAs; mm-critical ones first)
            x16g = sb.tile([128, 4, R], f16, tag="x16g")
            nc.sync.dma_start(out=x16g, in_=xT16.rearrange("(g p) r -> p g r", p=128))
            ws16g = sb.tile([128, 4, F], f16, tag="ws16g")
            nc.sync.dma_start(out=ws16g, in_=wsa16.rearrange("(g p) c -> p g c", p=128))
            xg = sb.tile([128, 4, R], f32, tag="xg")
            nc.sync.dma_start(out=xg, in_=xT.rearrange("(g p) r -> p g r", p=128))
            wscg = sb.tile([128, 4, 6 * H], f32, tag="wscg")
            nc.sync.dma_start(out=wscg, in_=wsc.rearrange("(g p) c -> p g c", p=128))

            evt = ob.tile([128, 4, 6 * H], f32, tag="evt", name="evt")
            for nb in range(IC):
                whp = ps.tile([128, 8, NH], f32, tag="whp")
                svp = ps.tile([128, 6 * H], f32, tag="svp")
                for fc in range(4):
                    nc.tensor.matmul(
                        whp, x16g[:, fc, nb * 128:(nb + 1) * 128], ws16g[:, fc, :],
                        start=(fc == 0), stop=(fc == 3),
                    )
                for fc in range(4):
                    nc.tensor.matmul(
                        svp, xg[:, fc, nb * 128:(nb + 1) * 128], wscg[:, fc, :],
                        start=(fc == 0), stop=(fc == 3),
                    )
                # [128, 8, 65] staging: ones col preset, one strided copy
                wt = ob.tile([128, 8, HC], f16, tag="wt")
                nc.vector.memset(wt[:, :, NH:HC], 1.0 / 64.0)
                nc.vector.tensor_copy(wt[:, :, 0:NH], whp)
                nc.sync.dma_start(out=wha16[nb * 128:(nb + 1) * 128, :], in_=wt)

                nc.scalar.activation(evt[:, nb, 0:4 * H], svp[:, 0:4 * H], AF.Exp)
                nc.vector.tensor_copy(evt[:, nb, 4 * H:6 * H], svp[:, 4 * H:6 * H])
            nc.sync.dma_start(
                out=ev.rearrange("(g p) c -> p g c", p=128), in_=evt
            )
    nc.compile()
    return nc


# ---------------------------------------------------------------- k1
K1_A = 52  # blocks routed via ACT (PE additive mask + Prelu/Exp); h5 + part h6
K1_Q = 52  # blocks routed via GPSIMD/Pool; h7 + rest of h6 + rest of h4
# remaining 256-A-Q blocks on DVE: h0..h3 + part of h4
K1_D = 256 - K1_A - K1_Q


def _quadify(jbs):
    """Split a jb list into group-aligned runs of 4 plus single leftovers."""
    out, i = [], 0
    jbs = list(jbs)
    while i < len(jbs):
        j = jbs[i]
        if j % 4 == 0 and jbs[i:i + 4] == [j, j + 1, j + 2, j + 3]:
            out.append(jbs[i:i + 4])
            i += 4
        else:
            out.append([j])
            i += 1
    return out


def _k1_assignment():
    d = K1_D
    assert 128 <= d <= 160 and 32 <= K1_A <= 64
    # DVE works head PAIRS interleaved by jb-quad so each arriving DMA group
    # offers ~4.5us of DVE work (single-head order starves on the loads)
    dve = []
    for h0 in (0, 2):
        for jq in range(JB // 4):
            for h in (h0, h0 + 1):
                dve.append((h, list(range(4 * jq, 4 * jq + 4))))
    dve += [(4, q) for q in _quadify(range(d - 128))]
    act = [(5, [jb]) for jb in range(JB)] + [(6, [jb]) for jb in range(K1_A - 32)]
    pool = (
        [(7, q) for q in _quadify(range(JB))]
        + [(6, q) for q in _quadify(range(K1_A - 32, JB))]
        + [(4, q) for q in _quadify(range(d - 128, JB))]
    )
    streams = {"dve": dve, "act": act, "pool": pool}
    act_heads = {h for (h, _) in act}
    scaled_heads = {h for (h, _) in dve} | {h for (h, _) in pool}
    return streams, act_heads, scaled_heads


def _unit_cost(e, n):
    if e == "dve":
        return n * 239.0 + (n * 512 * 0.5208 + 156.0)
    if e == "pool":
        return n * 868.0 + (n * 512 * 1.984 + 160.0)
    return 1662.0 * n


def _build_k1():
    """Per-core: 8-head attention for this core's R query rows + Who projection.

    Masked-softmax blocks routed across three engines in proportion to their
    cost-model throughput:
      DVE/Pool route (row-scaled by exp(-s1[i])):
        w = max(f1b*F2c, E2c) (tensor_scalar), p = w*adjT (tensor_tensor over
        a whole group of 4 key blocks at once - amortizes the per-op ramp)
      ACT route (unscaled): e' = (s1[i]-BIG) + BIG*adjT on PE,
        p = exp(lrelu(e'+s2[j])) via Prelu+Exp on ACT.
    outT[h] = sum_jb whaT_h @ p accumulates transposed [65, R]; the ones col
    (valued 1/64) gives rowsum/64 in row 64.  Finalize stays transposed:
    rowsum row -> f16 -> PE broadcast -> tensor divide -> ELU, leaving
    hcatT[h] [64, R] f16 which feeds Who matmuls as 64-partition stationaries
    (no transposes anywhere).  A head whose blocks span the ACT route and a
    scaled route keeps two psum accumulators, merged with exp(-s1) in
    finalize.  All broadcast rows (f1b, s1-BIG, exp(-s1)) are replicated on
    the host from k0's ev output - layout-only prep, no host transcendentals.

    in:  wha  [N, H*HC] f16 (full, from k0; ones cols are 1/64)
         evf  [N, 6H]  f32 (full)
         f1ball [128, H, R] f16, s1rball [1, H, R] f16, e1ball [65, R] f16
         adjt [N, R]   f16 (adj[my rows, :]^T, host prep)
         woa [F, F] f16, wosv16 [F, 2] f16
    out: whoa16 [R, F+1] f16 (hcat@Wo + ones col), svo [R, 3] f32
         (s1o, s2o, exp(-0.8 s1o))
    """
    nc = bacc.Bacc("TRN2", target_bir_lowering=False, debug=False, num_devices=M)
    wha = nc.dram_tensor("wha", [N, H * HC], f16, kind="ExternalInput").ap()
    evf = nc.dram_tensor("evf", [N, 6 * H], f32, kind="ExternalInput").ap()
    f1ball = nc.dram_tensor("f1ball", [128, H, R], f16, kind="ExternalInput").ap()
    s1rball = nc.dram_tensor("s1rball", [1, H, R], f16, kind="ExternalInput").ap()
    e1ball = nc.dram_tensor("e1ball", [65, R], f16, kind="ExternalInput").ap()
    adjt = nc.dram_tensor("adjt", [N, R], f16, kind="ExternalInput").ap()
    woa = nc.dram_tensor("woa", [F, F], f16, kind="ExternalInput").ap()
    wosv16 = nc.dram_tensor("wosv16", [F, 2], f16, kind="ExternalInput").ap()
    whoa16 = nc.dram_tensor("whoa16", [R, F + 1], f16, kind="ExternalOutput").ap()
    svo = nc.dram_tensor("svo", [R, 3], f32, kind="ExternalOutput").ap()

    streams, act_heads, scaled_heads = _k1_assignment()
    straddle_heads = act_heads & scaled_heads
    assert len(straddle_heads) <= 1
    expected = {}
    for eng, units in streams.items():
        for (h, jbs) in units:
            key = (h, "act" if eng == "act" else "sc")
            expected[key] = expected.get(key, 0) + len(jbs)

    with tile.TileContext(nc) as tc:
        with (
            tc.tile_pool(name="sb", bufs=1) as sb,
            tc.tile_pool(name="work", bufs=6) as work,
            tc.tile_pool(name="pw", bufs=14) as pw,
        ):
            ident = sb.tile([128, 128], f32, tag="ident")
            make_identity(nc, ident)
            ones16 = sb.tile([1, 128], f16, tag="ones16")
            nc.vector.memset(ones16, 1.0)
            ones65 = sb.tile([65, 64], f16, tag="ones65")
            nc.vector.memset(ones65, 1.0)
            bigi = sb.tile([128, 128], f16, tag="bigi")
            nc.vector.tensor_scalar(bigi, ident, BIG, None, op0=OP.mult)

            # --- resident loads: SP queue carries evb+adjt, ACT queue carries
            # the broadcast tables + wha + Wo so first groups land early ---
            GB = 4                      # jb blocks per DMA group
            NG = JB // GB               # 8 groups
            adjt_g = adjt.rearrange("(g b p) r -> g p b r", b=GB, p=128)
            wha_g = wha.rearrange("(g b p) c -> g p b c", b=GB, p=128)
            evb = sb.tile([128, JB, 6 * H], f32, tag="evb")
            nc.sync.dma_start(
                out=evb, in_=evf.rearrange("(b p) c -> p b c", p=128)
            )
            f1bt = sb.tile([128, H, R], f16, tag="f1bt")
            nc.scalar.dma_start(out=f1bt, in_=f1ball)
            s1rt = sb.tile([1, H, R], f16, tag="s1rt")
            nc.scalar.dma_start(out=s1rt, in_=s1rball)
            e1bt = sb.tile([65, R], f16, tag="e1bt")
            nc.scalar.dma_start(out=e1bt, in_=e1ball)
            woag = sb.tile([64, H, F], f16, tag="woag")
            nc.scalar.dma_start(out=woag, in_=woa.rearrange("(h p) c -> p h c", p=64))
            wosvg = sb.tile([64, H, 2], f16, tag="wosvg")
            nc.scalar.dma_start(out=wosvg, in_=wosv16.rearrange("(h p) c -> p h c", p=64))
            adjtb, whab = [], []
            for g in range(NG):
                t = sb.tile([128, GB, R], f16, tag=f"adjtb{g}", name=f"adjtb{g}")
                nc.sync.dma_start(out=t, in_=adjt_g[g])
                adjtb.append(t)
                t = sb.tile([128, GB, H * HC], f16, tag=f"whab{g}", name=f"whab{g}")
                nc.sync.dma_start(out=t, in_=wha_g[g])
                whab.append(t)
            adjts = [adjtb[jb // GB][:, jb % GB, :] for jb in range(JB)]
            whas = [whab[jb // GB][:, jb % GB, :] for jb in range(JB)]
            evs = [evb[:, jb, :] for jb in range(JB)]
            f1bs = {h: f1bt[:, h, :] for h in range(H)}
            s1rbs = {h: s1rt[:, h, :] for h in range(H)}

            hcTs = [
                sb.tile([64, R], f16, tag=f"hcT{h}", name=f"hcT{h}")
                for h in range(H)
            ]
            rs16v = sb.tile([65, R], f16, tag="rs16v")
            rcp16 = sb.tile([65, R], f16, tag="rcp16")

            with (
                tc.tile_pool(name="ap", bufs=1, space="PSUM") as accp,
                tc.tile_pool(name="fz", bufs=1, space="PSUM") as fzp,
            ):
                outts = {}
                counts = {}
                pending = {"dve": [], "act": [], "pool": []}
                PEND_DEPTH = {"dve": 1, "act": 1, "pool": 1}

                # static psum bank plan: heads sharing a tag have disjoint
                # accumulation lifetimes, so no ring wait can convoy PE
                BANK_TAG = {
                    (0, "sc"): "tA", (3, "sc"): "tA",
                    (1, "sc"): "tB", (6, "sc"): "tB",
                    (2, "sc"): "tC", (4, "sc"): "tC",
                    (7, "sc"): "tD",
                    (5, "act"): "tE", (6, "act"): "tF",
                    (5, "sc"): "tD", (4, "act"): "tF", (7, "act"): "tE",
                }

                def get_outt(h, cls):
                    key = (h, cls)
                    if key not in outts:
                        outts[key] = accp.tile(
                            [HC, R], f32, tag=BANK_TAG[key],
                            name=f"outt_{cls}{h}", bufs=1,
                        )
                        counts[key] = 0
                    return outts[key]

                def emit_vmm(h, cls, jb, p):
                    outt = get_outt(h, cls)
                    counts[(h, cls)] += 1
                    nc.tensor.matmul(
                        outt, whas[jb][:, h * HC:(h + 1) * HC], p,
                        start=(counts[(h, cls)] == 1),
                        stop=(counts[(h, cls)] == expected[(h, cls)]),
                    )

                def flush(e, all_=True):
                    while pending[e] and (all_ or len(pending[e]) > PEND_DEPTH[e]):
                        for args in pending[e].pop(0):
                            emit_vmm(*args)

                def emit_scaled(ename, h, jbs):
                    # n tensor_scalars (one per block: the per-partition
                    # scalars differ per key block) + ONE wide mask multiply
                    # over the whole aligned run; value matmuls deferred one
                    # unit so PE never waits on an in-flight p
                    eng = {"dve": nc.vector, "pool": nc.gpsimd}[ename]
                    n = len(jbs)
                    g, b0 = jbs[0] // GB, jbs[0] % GB
                    w = pw.tile([128, n, R], f16, tag=f"w{n}_{ename}", bufs=2 if n == 4 else 4)
                    for k, jb in enumerate(jbs):
                        eng.tensor_scalar(
                            w[:, k, :], f1bs[h], evs[jb][:, H + h:H + h + 1],
                            evs[jb][:, h:h + 1], op0=OP.mult, op1=OP.max,
                        )
                    p = pw.tile([128, n, R], f16, tag=f"p{n}_{ename}", bufs=3 if n == 4 else 4)
                    eng.tensor_tensor(
                        p, w, adjtb[g][:, b0:b0 + n, :], op=OP.mult
                    )
                    pending[ename].append(
                        [(h, "sc", jb, p[:, k, :]) for k, jb in enumerate(jbs)]
                    )
                    flush(ename, all_=False)

                def emit_act(h, jbs):
                    (jb,) = jbs
                    eps = fzp.tile([128, R], f32, tag="eps", bufs=1)
                    nc.tensor.matmul(eps, ones16, s1rbs[h], start=True, stop=False)
                    nc.tensor.matmul(eps, bigi, adjts[jb], start=False, stop=True)
                    m = pw.tile([128, R], f16, tag="m", bufs=4)
                    nc.scalar.activation(
                        m, eps, AF.Prelu, alpha=ALPHA,
                        bias=evb[:, jb, 5 * H + h:5 * H + h + 1],
                    )
                    p = pw.tile([128, R], f16, tag="p2", bufs=5)
                    nc.scalar.activation(p, m, AF.Exp)
                    pending["act"].append([(h, "act", jb, p)])
                    flush("act", all_=False)

                fin_pending = []

                def finalize_a(h, fin_eng):
                    # transposed finalize.  HW allows only ONE psum input per
                    # vector op, so the recip broadcast is copied to SBUF
                    # (on the finishing stream's engine) before the multiply.
                    if h in straddle_heads:
                        osc, oac = outts[(h, "sc")], outts[(h, "act")]
                        cmb = pw.tile([65, R], f16, tag="cmb", bufs=1)
                        nc.vector.tensor_tensor(cmb, oac, e1bt, op=OP.mult)
                        nc.vector.tensor_tensor(
                            rs16v[64:65, :], osc[64:65, :], cmb[64:65, :], op=OP.add
                        )
                        tq = pw.tile([64, R], f16, tag="tq", bufs=2)
                        nc.vector.scalar_tensor_tensor(
                            tq, osc[0:64, :], 0.0, cmb[0:64, :],
                            op0=OP.add, op1=OP.add,
                        )
                        num = tq
                        with nc.allow_low_precision(reason="softmax recip row"):
                            nc.vector.reciprocal(rcp16[64:65, :], rs16v[64:65, :])
                    else:
                        osc = outts.get((h, "sc"))
                        if osc is None:
                            osc = outts[(h, "act")]
                        num = osc[0:64, :]
                        with nc.allow_low_precision(reason="softmax recip row"):
                            nc.vector.reciprocal(rcp16[64:65, :], osc[64:65, :])
                    rb = fzp.tile([64, R], f32, tag="rb")
                    nc.tensor.matmul(
                        rb, ones65[64:65, :], rcp16[64:65, :], start=True, stop=True
                    )
                    rb16 = pw.tile([64, R], f16, tag="rb16", bufs=2)
                    nc.vector.tensor_copy(rb16, rb)
                    t = pw.tile([64, R], f16, tag="t", bufs=2)
                    nc.vector.tensor_tensor(t, num, rb16, op=OP.mult)
                    m0 = pw.tile([64, R], f16, tag="m0", bufs=2)
                    nc.vector.tensor_scalar(
                        m0, t, 1.0 / 64.0, 0.0, op0=OP.mult, op1=OP.min
                    )
                    ex = pw.tile([64, R], f16, tag="ex", bufs=3)
                    nc.scalar.activation(ex, m0, AF.Exp)
                    rl2 = pw.tile([64, R], f16, tag="rl2", bufs=3)
                    nc.vector.tensor_scalar(
                        rl2, t, 1.0 / 64.0, 0.0, op0=OP.mult, op1=OP.max
                    )
                    fin_pending.append([0, h, ex, rl2])

                def fin_tick(force=False):
                    for item in list(fin_pending):
                        item[0] += 1
                        if force or item[0] > 2:
                            _, h, ex, rl2 = item
                            nc.vector.tensor_tensor(hcTs[h], ex, rl2, op=OP.add)
                            nc.vector.tensor_scalar(
                                hcTs[h], hcTs[h], -1.0, None, op0=OP.add
                            )
                            fin_pending.remove(item)

                # --- merged emission by virtual engine clocks ---
                # finalize is deferred 2 units behind the stream that emitted
                # the head's last block: the engine executes behind emission,
                # and an early inline finalize stalls DVE's in-order stream
                fin_a_pending = {"dve": [], "act": [], "pool": []}

                FIN_DEPTH = {"dve": 2, "act": 2, "pool": 2}

                def fin_a_tick(e, force=False):
                    for item in list(fin_a_pending[e]):
                        item[0] += 1
                        if force or item[0] > FIN_DEPTH[e]:
                            finalize_a(item[1], e)
                            clocks["dve"] += 2300.0
                            clocks["act"] += 831.0
                            fin_a_pending[e].remove(item)

                clocks = {"dve": 0.0, "act": 0.0, "pool": 0.0}
                pos = {e: 0 for e in streams}
                head_done = {h: 0 for h in range(H)}
                remaining = sum(len(u) for u in streams.values())
                while remaining:
                    cand = []
                    for e in streams:
                        if pos[e] < len(streams[e]):
                            n = len(streams[e][pos[e]][1])
                            cand.append((clocks[e] + _unit_cost(e, n), e))
                    _, e = min(cand)
                    h, jbs = streams[e][pos[e]]
                    pos[e] += 1
                    remaining -= 1
                    clocks[e] += _unit_cost(e, len(jbs))
                    if e == "act":
                        emit_act(h, jbs)
                    else:
                        emit_scaled(e, h, jbs)
                    head_done[h] += len(jbs)
                    fin_tick()
                    fin_a_tick(e)
                    if pos[e] == len(streams[e]):
                        for e2 in ("dve", "act", "pool"):
                            flush(e2)
                        fin_a_tick(e, force=True)
                    if head_done[h] == JB:
                        for e2 in ("dve", "act", "pool"):
                            flush(e2)
                        fin_a_pending[e].append([0, h])
                for e2 in ("dve", "act", "pool"):
                    flush(e2)
                for e in fin_a_pending:
                    fin_a_tick(e, force=True)
                fin_tick(force=True)

            # --- Who = hcat @ [Wo | w1 | w2] for my rows (64-part stationaries)
            with tc.tile_pool(name="fp2", bufs=2, space="PSUM") as fp2:
                for ic in range(IC):
                    wop = fp2.tile([128, F], f32, tag="wop")
                    svp = fp2.tile([128, 2], f32, tag="svp2")
                    for h in range(H):
                        nc.tensor.matmul(
                            wop, hcTs[h][:, ic * 128:(ic + 1) * 128], woag[:, h, :],
                            start=(h == 0), stop=(h == H - 1),
                        )
                    for h in range(H):
                        nc.tensor.matmul(
                            svp, hcTs[h][:, ic * 128:(ic + 1) * 128], wosvg[:, h, :],
                            start=(h == 0), stop=(h == H - 1),
                        )
                    wt = work.tile([128, F + 1], f16, tag="wt")
                    nc.scalar.activation(wt[:, 0:F], wop, AF.Copy)
                    nc.vector.memset(wt[:, F:F + 1], 1.0)
                    nc.sync.dma_start(
                        out=whoa16[ic * 128:(ic + 1) * 128, :], in_=wt
                    )
                    st = work.tile([128, 3], f32, tag="st")
                    nc.vector.tensor_copy(st[:, 0:2], svp)
                    nc.scalar.activation(st[:, 2:3], svp[:, 0:1], AF.Exp, scale=-0.8)
                    nc.sync.dma_start(out=svo[ic * 128:(ic + 1) * 128, :], in_=st)
    nc.compile()
    return nc


# ---------------------------------------------------------------- k2
def _build_k2():
    """Per-core: output-layer attention for this core's R rows, final ELU.

    in:  whoa [N, F+1] f16, svof [N, 2] f32, svomy [R, 2] f32, adjt [N, R] f16
    out: out [R, F] f32
    """
    nc = bacc.Bacc("TRN2", target_bir_lowering=False, debug=False, num_devices=M)
    whoa = nc.dram_tensor("whoa", [N, F + 1], f16, kind="ExternalInput").ap()
    svof = nc.dram_tensor("svof", [N, 2], f32, kind="ExternalInput").ap()
    svomy = nc.dram_tensor("svomy", [R, 2], f32, kind="ExternalInput").ap()
    adjt = nc.dram_tensor("adjt", [N, R], f16, kind="ExternalInput").ap()
    out = nc.dram_tensor("out", [R, F], f32, kind="ExternalOutput").ap()

    with tile.TileContext(nc) as tc:
        with (
            tc.tile_pool(name="sb", bufs=1) as sb,
            tc.tile_pool(name="work", bufs=10) as work,
        ):
            # --- prep first (small DMAs ahead of the big resident loads) ---
            ident = sb.tile([128, 128], f32, tag="ident")
            make_identity(nc, ident)
            ones1 = sb.tile([1, 128], f32, tag="ones1")
            nc.vector.memset(ones1, 1.0)

            # blocked s2 [128, 32]: col b = s2o[b*128 + p]
            s2blk = sb.tile([128, JB], f32, tag="s2blk")
            nc.sync.dma_start(
                out=s2blk, in_=svof.rearrange("(b p) c -> p b c", p=128)[:, :, 1]
            )
            e2c = sb.tile([128, JB], f32, tag="e2c")
            f2c = sb.tile([128, JB], f32, tag="f2c")
            f1bo = sb.tile([128, R], f16, tag="f1bo")

            with tc.tile_pool(name="pp", bufs=2, space="PSUM") as pp:
                # global max of s2o -> stability shift: bias = 9 - max(s2o)
                mx1 = work.tile([128, 1], f32, tag="mx1")
                nc.vector.tensor_reduce(mx1, s2blk, axis=mybir.AxisListType.X, op=OP.max)
                mxp = pp.tile([1, 128], f32, tag="mxp")
                nc.tensor.transpose(mxp, mx1, ident)
                mxs = work.tile([1, 128], f32, tag="mxs")
                nc.vector.tensor_copy(mxs, mxp)
                mx2 = work.tile([1, 1], f32, tag="mx2")
                nc.vector.tensor_reduce(mx2, mxs, axis=mybir.AxisListType.X, op=OP.max)
                bias1 = work.tile([1, 1], f32, tag="bias1")
                nc.vector.tensor_scalar(
                    bias1, mx2, -1.0, 9.0, op0=OP.mult, op1=OP.add
                )
                biasb = sb.tile([128, 1], f32, tag="biasb")
                bp = pp.tile([128, 1], f32, tag="bp")
                nc.tensor.matmul(bp, ones1, bias1, start=True, stop=True)
                nc.vector.tensor_copy(biasb, bp)
                nc.scalar.activation(e2c, s2blk, AF.Exp, bias=biasb)
                nc.scalar.activation(f2c, s2blk, AF.Exp, bias=biasb, scale=0.2)

                # F1' broadcast tile from my s1o
                s1row = sb.tile([1, R], f32, tag="s1row")
                nc.sync.dma_start(
                    out=s1row, in_=svomy[:, 0:1].rearrange("r one -> one r")
                )
                f1row = work.tile([1, R], f32, tag="f1row")
                nc.scalar.activation(f1row, s1row, AF.Exp, scale=-0.8)
                fbp = pp.tile([128, R], f32, tag="fbp")
                nc.tensor.matmul(fbp, ones1, f1row, start=True, stop=True)
                nc.scalar.activation(f1bo, fbp, AF.Copy)

            # --- resident loads, coalesced grouped 3D-AP DMAs ---
            GB = 8
            NG = JB // GB
            adjt_g = adjt.rearrange("(g b p) r -> g p b r", b=GB, p=128)
            whoa_g = whoa.rearrange("(g b p) c -> g p b c", b=GB, p=128)
            adjtb, whob = [], []
            for g in range(NG):
                t = sb.tile([128, GB, R], f16, tag=f"adjtb{g}", name=f"adjtb{g}")
                nc.sync.dma_start(out=t, in_=adjt_g[g])
                adjtb.append(t)
                t = sb.tile([128, GB, F + 1], f16, tag=f"whob{g}", name=f"whob{g}")
                nc.sync.dma_start(out=t, in_=whoa_g[g])
                whob.append(t)
            adjts = [adjtb[jb // GB][:, jb % GB, :] for jb in range(JB)]
            whos = [whob[jb // GB][:, jb % GB, :] for jb in range(JB)]

            # --- main loop ---
            with tc.tile_pool(name="ap", bufs=1, space="PSUM") as accp:
                accs = [accp.tile([128, F], f32, tag=f"acc{ic}", name=f"acc{ic}") for ic in range(IC)]
                rss = [accp.tile([128, 1], f32, tag=f"rs{ic}", name=f"rs{ic}") for ic in range(IC)]
                SP = 384  # DVE takes [0:SP), GPSIMD [SP:R) - parallel halves
                for jb in range(JB):
                    w = work.tile([128, R], f16, tag="w")
                    p = work.tile([128, R], f16, tag="p")
                    nc.vector.tensor_scalar(
                        w[:, 0:SP], f1bo[:, 0:SP], f2c[:, jb:jb + 1],
                        e2c[:, jb:jb + 1], op0=OP.mult, op1=OP.max,
                    )
                    nc.vector.tensor_tensor(
                        p[:, 0:SP], w[:, 0:SP], adjts[jb][:, 0:SP], op=OP.mult
                    )
                    nc.gpsimd.tensor_scalar(
                        w[:, SP:R], f1bo[:, SP:R], f2c[:, jb:jb + 1],
                        e2c[:, jb:jb + 1], op0=OP.mult, op1=OP.max,
                    )
                    nc.gpsimd.tensor_tensor(
                        p[:, SP:R], w[:, SP:R], adjts[jb][:, SP:R], op=OP.mult
                    )
                    for ic in range(IC):
                        nc.tensor.matmul(
                            accs[ic], p[:, ic * 128:(ic + 1) * 128],
                            whos[jb][:, 0:F],
                            start=(jb == 0), stop=(jb == JB - 1),
                        )
                        nc.tensor.matmul(
                            rss[ic], p[:, ic * 128:(ic + 1) * 128],
                            whos[jb][:, F:F + 1],
                            start=(jb == 0), stop=(jb == JB - 1),
                        )
                for ic in range(IC):
                    r = work.tile([128, 1], f32, tag="r")
                    nc.vector.reciprocal(r, rss[ic])
                    ot = work.tile([128, F], f32, tag="ot")
                    nc.scalar.activation(ot, accs[ic], AF.Copy, scale=r)
                    ex = work.tile([128, F], f32, tag="ex")
                    nc.scalar.activation(ex, ot, AF.Exp)
                    rl = work.tile([128, F], f32, tag="rl")
                    nc.scalar.activation(rl, ot, AF.Relu)
                    nc.vector.scalar_tensor_tensor(
                        ot, ex, 1.0, rl, op0=OP.min, op1=OP.add
                    )
                    nc.vector.tensor_scalar(ot, ot, -1.0, None, op0=OP.add)
                    nc.sync.dma_start(out=out[ic * 128:(ic + 1) * 128, :], in_=ot)
    nc.compile()
    return nc


def _get(name):
    if name not in _CACHE:
        _CACHE[name] = {"k0": _build_k0, "k1": _build_k1, "k2": _build_k2}[name]()
    return _CACHE[name]


# ---------------------------------------------------------------- host
def kernel(x, left, adj, Ws, a1, a2, Wo, ao1, ao2):
    x = np.asarray(x, np.float32)
    adj = np.asarray(adj, np.float32)
    Ws = np.asarray(Ws, np.float32)
    a1 = np.asarray(a1, np.float32)
    a2 = np.asarray(a2, np.float32)
    Wo = np.asarray(Wo, np.float32)
    ao1 = np.asarray(ao1, np.float32)
    ao2 = np.asarray(ao2, np.float32)

    # host-side layout prep (no significant FLOPs)
    ws_all = np.ascontiguousarray(Ws.transpose(1, 0, 2).reshape(F, F))
    ws1 = np.einsum("hkf,hf->kh", Ws, a1)   # [F, H]  tiny matvecs
    ws2 = np.einsum("hkf,hf->kh", Ws, a2)
    wsc = np.ascontiguousarray(
        np.concatenate([ws2, 0.2 * ws2, -0.8 * ws1, -ws1, ws1, ws2], axis=1),
        dtype=np.float32,
    )
    wsa16 = ws_all.astype(np.float16)
    woa = np.ascontiguousarray(Wo).astype(np.float16)
    wosv16 = np.ascontiguousarray(
        np.stack([Wo @ ao1, Wo @ ao2], axis=1), dtype=np.float16
    )
    adj16 = adj.astype(np.float16)  # exact: adj is a 0/1 mask
    adjt_c = [
        np.ascontiguousarray(adj16[c * R:(c + 1) * R].T) for c in range(M)
    ]
    xt_c = [np.ascontiguousarray(x[c * R:(c + 1) * R].T) for c in range(M)]

    cores = list(range(M))

    k0 = _get("k0")
    res0 = _run(
        k0,
        [
            {
                "xT": xt_c[c],
                "xT16": xt_c[c].astype(np.float16),
                "wsa16": wsa16,
                "wsc": wsc,
            }
            for c in cores
        ],
        cores,
    )
    wha = np.concatenate([res0.results[c]["wha16"] for c in cores], axis=0)
    evf = np.concatenate([res0.results[c]["ev"] for c in cores], axis=0)

    # broadcast tables for k1, replicated (layout only) from k0's ev output
    streams, act_heads, scaled_heads = _k1_assignment()
    straddle = sorted(act_heads & scaled_heads)
    in1 = []
    for c in cores:
        evmy = evf[c * R:(c + 1) * R]
        f1ball = np.ascontiguousarray(np.broadcast_to(
            evmy[:, 2 * H:3 * H].T[None, :, :], (128, H, R)
        ).astype(np.float16))
        s1rball = np.ascontiguousarray(
            (evmy[:, 4 * H:5 * H].T[None, :, :] - BIG).astype(np.float16)
        )
        if straddle:
            e1ball = np.ascontiguousarray(np.broadcast_to(
                evmy[:, 3 * H + straddle[0]][None, :], (65, R)
            ).astype(np.float16))
        else:
            e1ball = np.zeros((65, R), np.float16)
        in1.append(
            {
                "wha": wha,
                "evf": evf,
                "f1ball": f1ball,
                "s1rball": s1rball,
                "e1ball": e1ball,
                "adjt": adjt_c[c],
                "woa": woa,
                "wosv16": wosv16,
            }
        )
    k1 = _get("k1")
    res1 = _run(k1, in1, cores)
    whoa = np.concatenate([res1.results[c]["whoa16"] for c in cores], axis=0)
    svof = np.concatenate([res1.results[c]["svo"] for c in cores], axis=0)

    svof2 = np.ascontiguousarray(svof[:, 0:2])
    k2 = _get("k2")
    res2 = _run(
        k2,
        [
            {
                "whoa": whoa,
                "svof": svof2,
                "svomy": svof2[c * R:(c + 1) * R],
                "adjt": adjt_c[c],
            }
            for c in cores
        ],
        cores,
    )
    return np.concatenate([res2.results[c]["out"] for c in cores], axis=0)


# revision 53
# speedup vs baseline: 1.0193x; 1.0193x over previous
"""GAT (graph attention network) forward pass on 8 Trainium2 NeuronCores.

Problem: nn_GAT - N=4096 nodes, F=512 features, H=8 heads, 1% dense adjacency.
    heads:  Wh = x @ Ws[h]; e = lrelu(s1[i]+s2[j]); att = masked softmax; elu(att @ Wh)
    out layer: same attention structure on hcat @ Wo, then elu.

Strategy (row-sharded across 8 cores, 3 launches):
  k0: each core computes Wh (all heads, fused matmul) + score vectors for its
      512 nodes; host gathers.
  k1: each core runs 8-head masked-softmax attention for its 512 query rows
      (key insight: exp(lrelu(e)) = max(exp(e), exp(0.2e)) and exp(e) factors
      rank-1 as exp(s1)[i]*exp(s2)[j], so the NxN tiles need NO transcendentals
      and NO PSUM e-matrix - just tensor_scalar/tensor_tensor ops in fp16.
      Softmax is invariant to per-query-row scaling, so scaling row i by
      exp(-s1[i]) turns the exp(e) branch into a per-partition scalar:
        u'[j,i] = exp(s2[j])          (tensor_scalar max)
        v'[j,i] = exp(-0.8*s1[i]) * exp(0.2*s2[j])   (tensor_scalar mult)
        p = max(u', v') * adjT        (mask multiply, fp16)
      The softmax denominator comes free as a ones-column in the value matrix.
      Blocks are routed across DVE, ACT (additive-mask PE+activation form) and
      GPSIMD in proportion to their simulated throughput.
      Also computes hcat @ Wo (+ output-layer score vectors) for its rows.
  k2: output-layer attention for the core's 512 rows; final ELU.

adj is passed from host as a pre-transposed fp16 (exact for a 0/1 mask) slice
per core; x is passed pre-transposed fp32 (pure layout prep, no FLOPs).
"""

import sys

for _p in ("/opt/trn_rl_repo",):
    if _p not in sys.path:
        sys.path.insert(0, _p)

import numpy as np

import concourse.bass as bass
import concourse.tile as tile
from concourse import bacc, mybir
from concourse.bass_utils import run_bass_kernel_spmd
from concourse.masks import make_identity

N, F, H, NH = 4096, 512, 8, 64
M = 8            # cores
R = N // M       # 512 query rows per core
JB = N // 128    # 32 key blocks
IC = R // 128    # 4 query-row chunks per core
HC = NH + 1      # 65 value cols per head (64 + ones col for row sums)
ALPHA = 0.2     # leaky relu slope
BIG = 200.0     # additive mask; 0.2*BIG=40 so masked exp underflows to 0
f32 = mybir.dt.float32
f16 = mybir.dt.float16
OP = mybir.AluOpType
AF = mybir.ActivationFunctionType

_CACHE = {}


def _run(nc, in_maps, core_ids, tries=3):
    """run_bass_kernel_spmd with retry: the axon-tunneled devices
    occasionally report NRT_EXEC_UNIT_UNRECOVERABLE transiently."""
    import time as _time

    for attempt in range(tries):
        try:
            return run_bass_kernel_spmd(nc, in_maps, core_ids=core_ids)
        except Exception:
            if attempt == tries - 1:
                raise
            _time.sleep(5.0)


# ---------------------------------------------------------------- k0
def _build_k0():
    """Per-core: Wh projection for this core's R nodes, all heads fused.

    in:  xT    [F, R] f32 / xT16 [F, R] f16 (this core's x rows, transposed)
         wsa16 [F, F] f16  (Ws stacked, head-major cols)
         wsc   [F, 6H] f32 (score cols: [ws2|.2ws2|-.8ws1|-ws1|ws1|ws2])
    out: wha16 [R, H*HC] f16 (per head: 64 value cols + ones col)
         ev    [R, 6H] f32 (exp(s2)|exp(.2s2)|exp(-.8s1)|exp(-s1)|s1|s2)
    """
    nc = bacc.Bacc("TRN2", target_bir_lowering=False, debug=False, num_devices=M)
    xT = nc.dram_tensor("xT", [F, R], f32, kind="ExternalInput").ap()
    xT16 = nc.dram_tensor("xT16", [F, R], f16, kind="ExternalInput").ap()
    wsa16 = nc.dram_tensor("wsa16", [F, F], f16, kind="ExternalInput").ap()
    wsc = nc.dram_tensor("wsc", [F, 6 * H], f32, kind="ExternalInput").ap()
    wha16 = nc.dram_tensor("wha16", [R, H * HC], f16, kind="ExternalOutput").ap()
    ev = nc.dram_tensor("ev", [R, 6 * H], f32, kind="ExternalOutput").ap()

    with tile.TileContext(nc) as tc:
        with (
            tc.tile_pool(name="sb", bufs=1) as sb,
            tc.tile_pool(name="ps", bufs=4, space="PSUM") as ps,
            tc.tile_pool(name="ob", bufs=4) as ob,
        ):
            # coalesced resident loads (few big DMAs; mm-critical ones first)
            x16g = sb.tile([128, 4, R], f16, tag="x16g")
            nc.sync.dma_start(out=x16g, in_=xT16.rearrange("(g p) r -> p g r", p=128))
            ws16g = sb.tile([128, 4, F], f16, tag="ws16g")
            nc.sync.dma_start(out=ws16g, in_=wsa16.rearrange("(g p) c -> p g c", p=128))
            xg = sb.tile([128, 4, R], f32, tag="xg")
            nc.sync.dma_start(out=xg, in_=xT.rearrange("(g p) r -> p g r", p=128))
            wscg = sb.tile([128, 4, 6 * H], f32, tag="wscg")
            nc.sync.dma_start(out=wscg, in_=wsc.rearrange("(g p) c -> p g c", p=128))

            evt = ob.tile([128, 4, 6 * H], f32, tag="evt", name="evt")
            for nb in range(IC):
                whp = ps.tile([128, 8, NH], f32, tag="whp")
                svp = ps.tile([128, 6 * H], f32, tag="svp")
                for fc in range(4):
                    nc.tensor.matmul(
                        whp, x16g[:, fc, nb * 128:(nb + 1) * 128], ws16g[:, fc, :],
                        start=(fc == 0), stop=(fc == 3),
                    )
                for fc in range(4):
                    nc.tensor.matmul(
                        svp, xg[:, fc, nb * 128:(nb + 1) * 128], wscg[:, fc, :],
                        start=(fc == 0), stop=(fc == 3),
                    )
                # [128, 8, 65] staging: ones col preset, one strided copy
                wt = ob.tile([128, 8, HC], f16, tag="wt")
                nc.vector.memset(wt[:, :, NH:HC], 1.0 / 64.0)
                nc.vector.tensor_copy(wt[:, :, 0:NH], whp)
                nc.sync.dma_start(out=wha16[nb * 128:(nb + 1) * 128, :], in_=wt)

                nc.scalar.activation(evt[:, nb, 0:4 * H], svp[:, 0:4 * H], AF.Exp)
                nc.vector.tensor_copy(evt[:, nb, 4 * H:6 * H], svp[:, 4 * H:6 * H])
            nc.sync.dma_start(
                out=ev.rearrange("(g p) c -> p g c", p=128), in_=evt
            )
    nc.compile()
    return nc


# ---------------------------------------------------------------- k1
K1_A = 52  # blocks routed via ACT (PE additive mask + Prelu/Exp); h5 + part h6
K1_Q = 52  # blocks routed via GPSIMD/Pool; h7 + rest of h6 + rest of h4
# remaining 256-A-Q blocks on DVE: h0..h3 + part of h4
K1_D = 256 - K1_A - K1_Q


def _quadify(jbs):
    """Split a jb list into group-aligned runs of 4 plus single leftovers."""
    out, i = [], 0
    jbs = list(jbs)
    while i < len(jbs):
        j = jbs[i]
        if j % 4 == 0 and jbs[i:i + 4] == [j, j + 1, j + 2, j + 3]:
            out.append(jbs[i:i + 4])
            i += 4
        else:
            out.append([j])
            i += 1
    return out


def _k1_assignment():
    d = K1_D
    assert 128 <= d <= 160 and 32 <= K1_A <= 64
    # DVE works head PAIRS interleaved by jb-quad so each arriving DMA group
    # offers ~4.5us of DVE work (single-head order starves on the loads)
    dve = []
    for h0 in (0, 2):
        for jq in range(JB // 4):
            for h in (h0, h0 + 1):
                dve.append((h, list(range(4 * jq, 4 * jq + 4))))
    # pool takes h7 + the tail of h4; DVE finishes h4's head and h6's
    # scaled tail so pool never touches the straddle head
    n4d = d - 128
    n6d = JB - (K1_A - 32)
    dve += [(4, q) for q in _quadify(range(n4d))]
    dve += [(6, q) for q in _quadify(range(K1_A - 32, JB))]
    act = [(5, [jb]) for jb in range(JB)] + [(6, [jb]) for jb in range(K1_A - 32)]
    pool = (
        [(7, q) for q in _quadify(range(JB))]
        + [(4, q) for q in _quadify(range(n4d, JB))]
    )
    streams = {"dve": dve, "act": act, "pool": pool}
    act_heads = {h for (h, _) in act}
    scaled_heads = {h for (h, _) in dve} | {h for (h, _) in pool}
    return streams, act_heads, scaled_heads


def _unit_cost(e, n):
    if e == "dve":
        return n * 239.0 + (n * 512 * 0.5208 + 156.0)
    if e == "pool":
        return n * 868.0 + (n * 512 * 1.984 + 160.0)
    return 1662.0 * n


def _build_k1():
    """Per-core: 8-head attention for this core's R query rows + Who projection.

    Masked-softmax blocks routed across three engines in proportion to their
    cost-model throughput:
      DVE/Pool route (row-scaled by exp(-s1[i])):
        w = max(f1b*F2c, E2c) (tensor_scalar), p = w*adjT (tensor_tensor over
        a whole group of 4 key blocks at once - amortizes the per-op ramp)
      ACT route (unscaled): e' = (s1[i]-BIG) + BIG*adjT on PE,
        p = exp(lrelu(e'+s2[j])) via Prelu+Exp on ACT.
    outT[h] = sum_jb whaT_h @ p accumulates transposed [65, R]; the ones col
    (valued 1/64) gives rowsum/64 in row 64.  Finalize stays transposed:
    rowsum row -> f16 -> PE broadcast -> tensor divide -> ELU, leaving
    hcatT[h] [64, R] f16 which feeds Who matmuls as 64-partition stationaries
    (no transposes anywhere).  A head whose blocks span the ACT route and a
    scaled route keeps two psum accumulators, merged with exp(-s1) in
    finalize.  All broadcast rows (f1b, s1-BIG, exp(-s1)) are replicated on
    the host from k0's ev output - layout-only prep, no host transcendentals.

    in:  wha  [N, H*HC] f16 (full, from k0; ones cols are 1/64)
         evf  [N, 6H]  f32 (full)
         f1ball [128, H, R] f16, s1rball [1, H, R] f16, e1ball [65, R] f16
         adjt [N, R]   f16 (adj[my rows, :]^T, host prep)
         woa [F, F] f16, wosv16 [F, 2] f16
    out: whoa16 [R, F+1] f16 (hcat@Wo + ones col), svo [R, 3] f32
         (s1o, s2o, exp(-0.8 s1o))
    """
    nc = bacc.Bacc("TRN2", target_bir_lowering=False, debug=False, num_devices=M)
    wha = nc.dram_tensor("wha", [N, H * HC], f16, kind="ExternalInput").ap()
    evf = nc.dram_tensor("evf", [N, 6 * H], f32, kind="ExternalInput").ap()
    f1ball = nc.dram_tensor("f1ball", [128, H, R], f16, kind="ExternalInput").ap()
    s1rball = nc.dram_tensor("s1rball", [1, H, R], f16, kind="ExternalInput").ap()
    e1ball = nc.dram_tensor("e1ball", [65, R], f16, kind="ExternalInput").ap()
    adjt = nc.dram_tensor("adjt", [N, R], f16, kind="ExternalInput").ap()
    woa = nc.dram_tensor("woa", [F, F], f16, kind="ExternalInput").ap()
    wosv16 = nc.dram_tensor("wosv16", [F, 2], f16, kind="ExternalInput").ap()
    whoa16 = nc.dram_tensor("whoa16", [R, F + 1], f16, kind="ExternalOutput").ap()
    svo = nc.dram_tensor("svo", [R, 3], f32, kind="ExternalOutput").ap()

    streams, act_heads, scaled_heads = _k1_assignment()
    straddle_heads = act_heads & scaled_heads
    assert len(straddle_heads) <= 1
    expected = {}
    for eng, units in streams.items():
        for (h, jbs) in units:
            key = (h, "act" if eng == "act" else "sc")
            expected[key] = expected.get(key, 0) + len(jbs)

    with tile.TileContext(nc) as tc:
        with (
            tc.tile_pool(name="sb", bufs=1) as sb,
            tc.tile_pool(name="work", bufs=6) as work,
            tc.tile_pool(name="pw", bufs=14) as pw,
        ):
            ident = sb.tile([128, 128], f32, tag="ident")
            make_identity(nc, ident)
            ones16 = sb.tile([1, 128], f16, tag="ones16")
            nc.vector.memset(ones16, 1.0)
            ones65 = sb.tile([65, 64], f16, tag="ones65")
            nc.vector.memset(ones65, 1.0)
            bigi = sb.tile([128, 128], f16, tag="bigi")
            nc.vector.tensor_scalar(bigi, ident, BIG, None, op0=OP.mult)

            # --- resident loads: SP queue carries evb+adjt, ACT queue carries
            # the broadcast tables + wha + Wo so first groups land early ---
            GB = 4                      # jb blocks per DMA group
            NG = JB // GB               # 8 groups
            adjt_g = adjt.rearrange("(g b p) r -> g p b r", b=GB, p=128)
            wha_g = wha.rearrange("(g b p) c -> g p b c", b=GB, p=128)
            evb = sb.tile([128, JB, 6 * H], f32, tag="evb")
            nc.sync.dma_start(
                out=evb, in_=evf.rearrange("(b p) c -> p b c", p=128)
            )
            f1bt = sb.tile([128, H, R], f16, tag="f1bt")
            nc.scalar.dma_start(out=f1bt, in_=f1ball)
            s1rt = sb.tile([1, H, R], f16, tag="s1rt")
            nc.scalar.dma_start(out=s1rt, in_=s1rball)
            e1bt = sb.tile([65, R], f16, tag="e1bt")
            nc.scalar.dma_start(out=e1bt, in_=e1ball)
            woag = sb.tile([64, H, F], f16, tag="woag")
            nc.scalar.dma_start(out=woag, in_=woa.rearrange("(h p) c -> p h c", p=64))
            wosvg = sb.tile([64, H, 2], f16, tag="wosvg")
            nc.scalar.dma_start(out=wosvg, in_=wosv16.rearrange("(h p) c -> p h c", p=64))
            adjtb, whab = [], []
            for g in range(NG):
                t = sb.tile([128, GB, R], f16, tag=f"adjtb{g}", name=f"adjtb{g}")
                nc.sync.dma_start(out=t, in_=adjt_g[g])
                adjtb.append(t)
                t = sb.tile([128, GB, H * HC], f16, tag=f"whab{g}", name=f"whab{g}")
                nc.sync.dma_start(out=t, in_=wha_g[g])
                whab.append(t)
            adjts = [adjtb[jb // GB][:, jb % GB, :] for jb in range(JB)]
            whas = [whab[jb // GB][:, jb % GB, :] for jb in range(JB)]
            evs = [evb[:, jb, :] for jb in range(JB)]
            f1bs = {h: f1bt[:, h, :] for h in range(H)}
            s1rbs = {h: s1rt[:, h, :] for h in range(H)}

            hcTs = [
                sb.tile([64, R], f16, tag=f"hcT{h}", name=f"hcT{h}")
                for h in range(H)
            ]
            rs16v = sb.tile([65, R], f16, tag="rs16v")
            rcp16 = sb.tile([65, R], f16, tag="rcp16")

            with (
                tc.tile_pool(name="ap", bufs=1, space="PSUM") as accp,
                tc.tile_pool(name="fz", bufs=1, space="PSUM") as fzp,
            ):
                outts = {}
                counts = {}
                pending = {"dve": [], "act": [], "pool": []}
                PEND_DEPTH = {"dve": 1, "act": 1, "pool": 1}

                # static psum bank plan: heads sharing a tag have disjoint
                # accumulation lifetimes, so no ring wait can convoy PE
                BANK_TAG = {
                    (0, "sc"): "tA", (3, "sc"): "tA",
                    (1, "sc"): "tB", (6, "sc"): "tB",
                    (2, "sc"): "tC", (4, "sc"): "tC",
                    (7, "sc"): "tD",
                    (5, "act"): "tE", (6, "act"): "tF",
                    (5, "sc"): "tD", (4, "act"): "tF", (7, "act"): "tE",
                }

                def get_outt(h, cls):
                    key = (h, cls)
                    if key not in outts:
                        outts[key] = accp.tile(
                            [HC, R], f32, tag=BANK_TAG[key],
                            name=f"outt_{cls}{h}", bufs=1,
                        )
                        counts[key] = 0
                    return outts[key]

                def emit_vmm(h, cls, jb, p):
                    outt = get_outt(h, cls)
                    counts[(h, cls)] += 1
                    nc.tensor.matmul(
                        outt, whas[jb][:, h * HC:(h + 1) * HC], p,
                        start=(counts[(h, cls)] == 1),
                        stop=(counts[(h, cls)] == expected[(h, cls)]),
                    )

                def flush(e, all_=True):
                    while pending[e] and (all_ or len(pending[e]) > PEND_DEPTH[e]):
                        for args in pending[e].pop(0):
                            emit_vmm(*args)

                def emit_scaled(ename, h, jbs):
                    # n tensor_scalars (one per block: the per-partition
                    # scalars differ per key block) + ONE wide mask multiply
                    # over the whole aligned run; value matmuls deferred one
                    # unit so PE never waits on an in-flight p
                    eng = {"dve": nc.vector, "pool": nc.gpsimd}[ename]
                    n = len(jbs)
                    g, b0 = jbs[0] // GB, jbs[0] % GB
                    w = pw.tile([128, n, R], f16, tag=f"w{n}_{ename}", bufs=2 if n == 4 else 4)
                    for k, jb in enumerate(jbs):
                        eng.tensor_scalar(
                            w[:, k, :], f1bs[h], evs[jb][:, H + h:H + h + 1],
                            evs[jb][:, h:h + 1], op0=OP.mult, op1=OP.max,
                        )
                    p = pw.tile([128, n, R], f16, tag=f"p{n}_{ename}", bufs=3 if n == 4 else 4)
                    eng.tensor_tensor(
                        p, w, adjtb[g][:, b0:b0 + n, :], op=OP.mult
                    )
                    pending[ename].append(
                        [(h, "sc", jb, p[:, k, :]) for k, jb in enumerate(jbs)]
                    )
                    flush(ename, all_=False)

                def emit_act(h, jbs):
                    (jb,) = jbs
                    eps = fzp.tile([128, R], f32, tag="eps", bufs=1)
                    nc.tensor.matmul(eps, ones16, s1rbs[h], start=True, stop=False)
                    nc.tensor.matmul(eps, bigi, adjts[jb], start=False, stop=True)
                    m = pw.tile([128, R], f16, tag="m", bufs=4)
                    nc.scalar.activation(
                        m, eps, AF.Prelu, alpha=ALPHA,
                        bias=evb[:, jb, 5 * H + h:5 * H + h + 1],
                    )
                    p = pw.tile([128, R], f16, tag="p2", bufs=5)
                    nc.scalar.activation(p, m, AF.Exp)
                    pending["act"].append([(h, "act", jb, p)])
                    flush("act", all_=False)

                fin_pending = []

                def finalize_a(h, fin_eng):
                    # transposed finalize.  HW allows only ONE psum input per
                    # vector op, so the recip broadcast is copied to SBUF
                    # (on the finishing stream's engine) before the multiply.
                    if h in straddle_heads:
                        osc, oac = outts[(h, "sc")], outts[(h, "act")]
                        cmb = pw.tile([65, R], f16, tag="cmb", bufs=1)
                        nc.vector.tensor_tensor(cmb, oac, e1bt, op=OP.mult)
                        nc.vector.tensor_tensor(
                            rs16v[64:65, :], osc[64:65, :], cmb[64:65, :], op=OP.add
                        )
                        tq = pw.tile([64, R], f16, tag="tq", bufs=2)
                        nc.vector.scalar_tensor_tensor(
                            tq, osc[0:64, :], 0.0, cmb[0:64, :],
                            op0=OP.add, op1=OP.add,
                        )
                        num = tq
                        with nc.allow_low_precision(reason="softmax recip row"):
                            nc.vector.reciprocal(rcp16[64:65, :], rs16v[64:65, :])
                    else:
                        osc = outts.get((h, "sc"))
                        if osc is None:
                            osc = outts[(h, "act")]
                        num = osc[0:64, :]
                        with nc.allow_low_precision(reason="softmax recip row"):
                            nc.vector.reciprocal(rcp16[64:65, :], osc[64:65, :])
                    rb = fzp.tile([64, R], f32, tag="rb")
                    nc.tensor.matmul(
                        rb, ones65[64:65, :], rcp16[64:65, :], start=True, stop=True
                    )
                    rb16 = pw.tile([64, R], f16, tag="rb16", bufs=2)
                    nc.vector.tensor_copy(rb16, rb)
                    t = pw.tile([64, R], f16, tag="t", bufs=2)
                    nc.vector.tensor_tensor(t, num, rb16, op=OP.mult)
                    m0 = pw.tile([64, R], f16, tag="m0", bufs=2)
                    nc.vector.tensor_scalar(
                        m0, t, 1.0 / 64.0, 0.0, op0=OP.mult, op1=OP.min
                    )
                    ex = pw.tile([64, R], f16, tag="ex", bufs=3)
                    nc.scalar.activation(ex, m0, AF.Exp)
                    rl2 = pw.tile([64, R], f16, tag="rl2", bufs=3)
                    nc.vector.tensor_scalar(
                        rl2, t, 1.0 / 64.0, 0.0, op0=OP.mult, op1=OP.max
                    )
                    fin_pending.append([0, h, ex, rl2])

                def fin_tick(force=False):
                    for item in list(fin_pending):
                        item[0] += 1
                        if force or item[0] > 2:
                            _, h, ex, rl2 = item
                            nc.vector.tensor_tensor(hcTs[h], ex, rl2, op=OP.add)
                            nc.vector.tensor_scalar(
                                hcTs[h], hcTs[h], -1.0, None, op0=OP.add
                            )
                            fin_pending.remove(item)

                # --- merged emission by virtual engine clocks ---
                # finalize is deferred 2 units behind the stream that emitted
                # the head's last block: the engine executes behind emission,
                # and an early inline finalize stalls DVE's in-order stream
                fin_a_pending = {"dve": [], "act": [], "pool": []}

                FIN_DEPTH = {"dve": 2, "act": 2, "pool": 2}

                def fin_a_tick(e, force=False):
                    for item in list(fin_a_pending[e]):
                        item[0] += 1
                        if force or item[0] > FIN_DEPTH[e]:
                            finalize_a(item[1], e)
                            clocks["dve"] += 2300.0
                            clocks["act"] += 831.0
                            fin_a_pending[e].remove(item)

                clocks = {"dve": 0.0, "act": 0.0, "pool": 0.0}
                pos = {e: 0 for e in streams}
                head_done = {h: 0 for h in range(H)}
                remaining = sum(len(u) for u in streams.values())
                while remaining:
                    cand = []
                    for e in streams:
                        if pos[e] < len(streams[e]):
                            n = len(streams[e][pos[e]][1])
                            cand.append((clocks[e] + _unit_cost(e, n), e))
                    _, e = min(cand)
                    h, jbs = streams[e][pos[e]]
                    pos[e] += 1
                    remaining -= 1
                    clocks[e] += _unit_cost(e, len(jbs))
                    if e == "act":
                        emit_act(h, jbs)
                    else:
                        emit_scaled(e, h, jbs)
                    head_done[h] += len(jbs)
                    fin_tick()
                    fin_a_tick(e)
                    if pos[e] == len(streams[e]):
                        for e2 in ("dve", "act", "pool"):
                            flush(e2)
                        fin_a_tick(e, force=True)
                    if head_done[h] == JB:
                        for e2 in ("dve", "act", "pool"):
                            flush(e2)
                        fin_a_pending[e].append([0, h])
                for e2 in ("dve", "act", "pool"):
                    flush(e2)
                for e in fin_a_pending:
                    fin_a_tick(e, force=True)
                fin_tick(force=True)

            # --- Who = hcat @ [Wo | w1 | w2] for my rows (64-part stationaries)
            with tc.tile_pool(name="fp2", bufs=2, space="PSUM") as fp2:
                for ic in range(IC):
                    wop = fp2.tile([128, F], f32, tag="wop")
                    svp = fp2.tile([128, 2], f32, tag="svp2")
                    for h in range(H):
                        nc.tensor.matmul(
                            wop, hcTs[h][:, ic * 128:(ic + 1) * 128], woag[:, h, :],
                            start=(h == 0), stop=(h == H - 1),
                        )
                    for h in range(H):
                        nc.tensor.matmul(
                            svp, hcTs[h][:, ic * 128:(ic + 1) * 128], wosvg[:, h, :],
                            start=(h == 0), stop=(h == H - 1),
                        )
                    wt = work.tile([128, F + 1], f16, tag="wt")
                    nc.scalar.activation(wt[:, 0:F], wop, AF.Copy)
                    nc.vector.memset(wt[:, F:F + 1], 1.0)
                    nc.sync.dma_start(
                        out=whoa16[ic * 128:(ic + 1) * 128, :], in_=wt
                    )
                    st = work.tile([128, 3], f32, tag="st")
                    nc.vector.tensor_copy(st[:, 0:2], svp)
                    nc.scalar.activation(st[:, 2:3], svp[:, 0:1], AF.Exp, scale=-0.8)
                    nc.sync.dma_start(out=svo[ic * 128:(ic + 1) * 128, :], in_=st)
    nc.compile()
    return nc


# ---------------------------------------------------------------- k2
def _build_k2():
    """Per-core: output-layer attention for this core's R rows, final ELU.

    in:  whoa [N, F+1] f16, svof [N, 2] f32, svomy [R, 2] f32, adjt [N, R] f16
    out: out [R, F] f32
    """
    nc = bacc.Bacc("TRN2", target_bir_lowering=False, debug=False, num_devices=M)
    whoa = nc.dram_tensor("whoa", [N, F + 1], f16, kind="ExternalInput").ap()
    svof = nc.dram_tensor("svof", [N, 2], f32, kind="ExternalInput").ap()
    svomy = nc.dram_tensor("svomy", [R, 2], f32, kind="ExternalInput").ap()
    adjt = nc.dram_tensor("adjt", [N, R], f16, kind="ExternalInput").ap()
    out = nc.dram_tensor("out", [R, F], f16, kind="ExternalOutput").ap()

    with tile.TileContext(nc) as tc:
        with (
            tc.tile_pool(name="sb", bufs=1) as sb,
            tc.tile_pool(name="work", bufs=10) as work,
        ):
            # --- prep first (small DMAs ahead of the big resident loads) ---
            ident = sb.tile([128, 128], f32, tag="ident")
            make_identity(nc, ident)
            ones1 = sb.tile([1, 128], f32, tag="ones1")
            nc.vector.memset(ones1, 1.0)

            # blocked s2 [128, 32]: col b = s2o[b*128 + p]
            s2blk = sb.tile([128, JB], f32, tag="s2blk")
            nc.sync.dma_start(
                out=s2blk, in_=svof.rearrange("(b p) c -> p b c", p=128)[:, :, 1]
            )
            e2c = sb.tile([128, JB], f32, tag="e2c")
            f2c = sb.tile([128, JB], f32, tag="f2c")
            f1bo = sb.tile([128, R], f16, tag="f1bo")

            with tc.tile_pool(name="pp", bufs=2, space="PSUM") as pp:
                # global max of s2o -> stability shift: bias = 9 - max(s2o)
                mx1 = work.tile([128, 1], f32, tag="mx1")
                nc.vector.tensor_reduce(mx1, s2blk, axis=mybir.AxisListType.X, op=OP.max)
                mxp = pp.tile([1, 128], f32, tag="mxp")
                nc.tensor.transpose(mxp, mx1, ident)
                mxs = work.tile([1, 128], f32, tag="mxs")
                nc.vector.tensor_copy(mxs, mxp)
                mx2 = work.tile([1, 1], f32, tag="mx2")
                nc.vector.tensor_reduce(mx2, mxs, axis=mybir.AxisListType.X, op=OP.max)
                bias1 = work.tile([1, 1], f32, tag="bias1")
                nc.vector.tensor_scalar(
                    bias1, mx2, -1.0, 9.0, op0=OP.mult, op1=OP.add
                )
                biasb = sb.tile([128, 1], f32, tag="biasb")
                bp = pp.tile([128, 1], f32, tag="bp")
                nc.tensor.matmul(bp, ones1, bias1, start=True, stop=True)
                nc.vector.tensor_copy(biasb, bp)
                nc.scalar.activation(e2c, s2blk, AF.Exp, bias=biasb)
                nc.scalar.activation(f2c, s2blk, AF.Exp, bias=biasb, scale=0.2)

                # F1' broadcast tile from my s1o
                s1row = sb.tile([1, R], f32, tag="s1row")
                nc.sync.dma_start(
                    out=s1row, in_=svomy[:, 0:1].rearrange("r one -> one r")
                )
                f1row = work.tile([1, R], f32, tag="f1row")
                nc.scalar.activation(f1row, s1row, AF.Exp, scale=-0.8)
                fbp = pp.tile([128, R], f32, tag="fbp")
                nc.tensor.matmul(fbp, ones1, f1row, start=True, stop=True)
                nc.scalar.activation(f1bo, fbp, AF.Copy)

            # --- resident loads, coalesced grouped 3D-AP DMAs ---
            GB = 8
            NG = JB // GB
            adjt_g = adjt.rearrange("(g b p) r -> g p b r", b=GB, p=128)
            whoa_g = whoa.rearrange("(g b p) c -> g p b c", b=GB, p=128)
            adjtb, whob = [], []
            for g in range(NG):
                t = sb.tile([128, GB, R], f16, tag=f"adjtb{g}", name=f"adjtb{g}")
                nc.sync.dma_start(out=t, in_=adjt_g[g])
                adjtb.append(t)
                t = sb.tile([128, GB, F + 1], f16, tag=f"whob{g}", name=f"whob{g}")
                nc.sync.dma_start(out=t, in_=whoa_g[g])
                whob.append(t)
            adjts = [adjtb[jb // GB][:, jb % GB, :] for jb in range(JB)]
            whos = [whob[jb // GB][:, jb % GB, :] for jb in range(JB)]

            # --- main loop ---
            with tc.tile_pool(name="ap", bufs=1, space="PSUM") as accp:
                accs = [accp.tile([128, F], f32, tag=f"acc{ic}", name=f"acc{ic}") for ic in range(IC)]
                rss = [accp.tile([128, 1], f32, tag=f"rs{ic}", name=f"rs{ic}") for ic in range(IC)]
                SP = 384  # DVE takes [0:SP), GPSIMD [SP:R) - parallel halves
                for jb in range(JB):
                    w = work.tile([128, R], f16, tag="w")
                    p = work.tile([128, R], f16, tag="p")
                    nc.vector.tensor_scalar(
                        w[:, 0:SP], f1bo[:, 0:SP], f2c[:, jb:jb + 1],
                        e2c[:, jb:jb + 1], op0=OP.mult, op1=OP.max,
                    )
                    nc.vector.tensor_tensor(
                        p[:, 0:SP], w[:, 0:SP], adjts[jb][:, 0:SP], op=OP.mult
                    )
                    nc.gpsimd.tensor_scalar(
                        w[:, SP:R], f1bo[:, SP:R], f2c[:, jb:jb + 1],
                        e2c[:, jb:jb + 1], op0=OP.mult, op1=OP.max,
                    )
                    nc.gpsimd.tensor_tensor(
                        p[:, SP:R], w[:, SP:R], adjts[jb][:, SP:R], op=OP.mult
                    )
                    for ic in range(IC):
                        nc.tensor.matmul(
                            accs[ic], p[:, ic * 128:(ic + 1) * 128],
                            whos[jb][:, 0:F],
                            start=(jb == 0), stop=(jb == JB - 1),
                        )
                        nc.tensor.matmul(
                            rss[ic], p[:, ic * 128:(ic + 1) * 128],
                            whos[jb][:, F:F + 1],
                            start=(jb == 0), stop=(jb == JB - 1),
                        )
                for ic in range(IC):
                    r = work.tile([128, 1], f32, tag="r")
                    nc.vector.reciprocal(r, rss[ic])
                    ot = work.tile([128, F], f16, tag="ot")
                    nc.scalar.activation(ot, accs[ic], AF.Copy, scale=r)
                    m0 = work.tile([128, F], f16, tag="m0")
                    nc.vector.tensor_scalar(m0, ot, 0.0, None, op0=OP.min)
                    ex = work.tile([128, F], f16, tag="ex")
                    nc.scalar.activation(ex, m0, AF.Exp)
                    rl2 = work.tile([128, F], f16, tag="rl2")
                    nc.vector.tensor_scalar(rl2, ot, 0.0, None, op0=OP.max)
                    res = work.tile([128, F], f16, tag="res")
                    nc.vector.tensor_tensor(res, ex, rl2, op=OP.add)
                    nc.vector.tensor_scalar(res, res, -1.0, None, op0=OP.add)
                    nc.sync.dma_start(out=out[ic * 128:(ic + 1) * 128, :], in_=res)
    nc.compile()
    return nc


def _get(name):
    if name not in _CACHE:
        _CACHE[name] = {"k0": _build_k0, "k1": _build_k1, "k2": _build_k2}[name]()
    return _CACHE[name]


# ---------------------------------------------------------------- host
def kernel(x, left, adj, Ws, a1, a2, Wo, ao1, ao2):
    x = np.asarray(x, np.float32)
    adj = np.asarray(adj, np.float32)
    Ws = np.asarray(Ws, np.float32)
    a1 = np.asarray(a1, np.float32)
    a2 = np.asarray(a2, np.float32)
    Wo = np.asarray(Wo, np.float32)
    ao1 = np.asarray(ao1, np.float32)
    ao2 = np.asarray(ao2, np.float32)

    # host-side layout prep (no significant FLOPs)
    ws_all = np.ascontiguousarray(Ws.transpose(1, 0, 2).reshape(F, F))
    ws1 = np.einsum("hkf,hf->kh", Ws, a1)   # [F, H]  tiny matvecs
    ws2 = np.einsum("hkf,hf->kh", Ws, a2)
    wsc = np.ascontiguousarray(
        np.concatenate([ws2, 0.2 * ws2, -0.8 * ws1, -ws1, ws1, ws2], axis=1),
        dtype=np.float32,
    )
    wsa16 = ws_all.astype(np.float16)
    woa = np.ascontiguousarray(Wo).astype(np.float16)
    wosv16 = np.ascontiguousarray(
        np.stack([Wo @ ao1, Wo @ ao2], axis=1), dtype=np.float16
    )
    adj16 = adj.astype(np.float16)  # exact: adj is a 0/1 mask
    adjt_c = [
        np.ascontiguousarray(adj16[c * R:(c + 1) * R].T) for c in range(M)
    ]
    xt_c = [np.ascontiguousarray(x[c * R:(c + 1) * R].T) for c in range(M)]

    cores = list(range(M))

    k0 = _get("k0")
    res0 = _run(
        k0,
        [
            {
                "xT": xt_c[c],
                "xT16": xt_c[c].astype(np.float16),
                "wsa16": wsa16,
                "wsc": wsc,
            }
            for c in cores
        ],
        cores,
    )
    wha = np.concatenate([res0.results[c]["wha16"] for c in cores], axis=0)
    evf = np.concatenate([res0.results[c]["ev"] for c in cores], axis=0)

    # broadcast tables for k1, replicated (layout only) from k0's ev output
    streams, act_heads, scaled_heads = _k1_assignment()
    straddle = sorted(act_heads & scaled_heads)
    in1 = []
    for c in cores:
        evmy = evf[c * R:(c + 1) * R]
        f1ball = np.ascontiguousarray(np.broadcast_to(
            evmy[:, 2 * H:3 * H].T[None, :, :], (128, H, R)
        ).astype(np.float16))
        s1rball = np.ascontiguousarray(
            (evmy[:, 4 * H:5 * H].T[None, :, :] - BIG).astype(np.float16)
        )
        if straddle:
            e1ball = np.ascontiguousarray(np.broadcast_to(
                evmy[:, 3 * H + straddle[0]][None, :], (65, R)
            ).astype(np.float16))
        else:
            e1ball = np.zeros((65, R), np.float16)
        in1.append(
            {
                "wha": wha,
                "evf": evf,
                "f1ball": f1ball,
                "s1rball": s1rball,
                "e1ball": e1ball,
                "adjt": adjt_c[c],
                "woa": woa,
                "wosv16": wosv16,
            }
        )
    k1 = _get("k1")
    res1 = _run(k1, in1, cores)
    whoa = np.concatenate([res1.results[c]["whoa16"] for c in cores], axis=0)
    svof = np.concatenate([res1.results[c]["svo"] for c in cores], axis=0)

    svof2 = np.ascontiguousarray(svof[:, 0:2])
    k2 = _get("k2")
    res2 = _run(
        k2,
        [
            {
                "whoa": whoa,
                "svof": svof2,
                "svomy": svof2[c * R:(c + 1) * R],
                "adjt": adjt_c[c],
            }
            for c in cores
        ],
        cores,
    )
    return np.concatenate(
        [res2.results[c]["out"] for c in cores], axis=0
    ).astype(np.float32)


# revision 57
# speedup vs baseline: 1.0401x; 1.0204x over previous
"""GAT (graph attention network) forward pass on 8 Trainium2 NeuronCores.

Problem: nn_GAT - N=4096 nodes, F=512 features, H=8 heads, 1% dense adjacency.
    heads:  Wh = x @ Ws[h]; e = lrelu(s1[i]+s2[j]); att = masked softmax; elu(att @ Wh)
    out layer: same attention structure on hcat @ Wo, then elu.

Strategy (row-sharded across 8 cores, 3 launches):
  k0: each core computes Wh (all heads, fused matmul) + score vectors for its
      512 nodes; host gathers.
  k1: each core runs 8-head masked-softmax attention for its 512 query rows
      (key insight: exp(lrelu(e)) = max(exp(e), exp(0.2e)) and exp(e) factors
      rank-1 as exp(s1)[i]*exp(s2)[j], so the NxN tiles need NO transcendentals
      and NO PSUM e-matrix - just tensor_scalar/tensor_tensor ops in fp16.
      Softmax is invariant to per-query-row scaling, so scaling row i by
      exp(-s1[i]) turns the exp(e) branch into a per-partition scalar:
        u'[j,i] = exp(s2[j])          (tensor_scalar max)
        v'[j,i] = exp(-0.8*s1[i]) * exp(0.2*s2[j])   (tensor_scalar mult)
        p = max(u', v') * adjT        (mask multiply, fp16)
      The softmax denominator comes free as a ones-column in the value matrix.
      Blocks are routed across DVE, ACT (additive-mask PE+activation form) and
      GPSIMD in proportion to their simulated throughput.
      Also computes hcat @ Wo (+ output-layer score vectors) for its rows.
  k2: output-layer attention for the core's 512 rows; final ELU.

adj is passed from host as a pre-transposed fp16 (exact for a 0/1 mask) slice
per core; x is passed pre-transposed fp32 (pure layout prep, no FLOPs).
"""

import sys

for _p in ("/opt/trn_rl_repo",):
    if _p not in sys.path:
        sys.path.insert(0, _p)

import numpy as np

import concourse.bass as bass
import concourse.tile as tile
from concourse import bacc, mybir
from concourse.bass_utils import run_bass_kernel_spmd
from concourse.masks import make_identity

N, F, H, NH = 4096, 512, 8, 64
M = 8            # cores
R = N // M       # 512 query rows per core
JB = N // 128    # 32 key blocks
IC = R // 128    # 4 query-row chunks per core
HC = NH + 1      # 65 value cols per head (64 + ones col for row sums)
ALPHA = 0.2     # leaky relu slope
BIG = 200.0     # additive mask; 0.2*BIG=40 so masked exp underflows to 0
f32 = mybir.dt.float32
f16 = mybir.dt.float16
OP = mybir.AluOpType
AF = mybir.ActivationFunctionType

_CACHE = {}


def _run(nc, in_maps, core_ids, tries=3):
    """run_bass_kernel_spmd with retry: the axon-tunneled devices
    occasionally report NRT_EXEC_UNIT_UNRECOVERABLE transiently."""
    import time as _time

    for attempt in range(tries):
        try:
            return run_bass_kernel_spmd(nc, in_maps, core_ids=core_ids)
        except Exception:
            if attempt == tries - 1:
                raise
            _time.sleep(5.0)


# ---------------------------------------------------------------- k0
def _build_k0():
    """Per-core: Wh projection for this core's R nodes, all heads fused.

    in:  xT    [F, R] f32 / xT16 [F, R] f16 (this core's x rows, transposed)
         wsa16 [F, F] f16  (Ws stacked, head-major cols)
         wsc   [F, 6H] f32 (score cols: [ws2|.2ws2|-.8ws1|-ws1|ws1|ws2])
    out: wha16 [R, H*HC] f16 (per head: 64 value cols + ones col)
         ev    [R, 6H] f32 (exp(s2)|exp(.2s2)|exp(-.8s1)|exp(-s1)|s1|s2)
    """
    nc = bacc.Bacc("TRN2", target_bir_lowering=False, debug=False, num_devices=M)
    xT = nc.dram_tensor("xT", [F, R], f32, kind="ExternalInput").ap()
    xT16 = nc.dram_tensor("xT16", [F, R], f16, kind="ExternalInput").ap()
    wsa16 = nc.dram_tensor("wsa16", [F, F], f16, kind="ExternalInput").ap()
    wsc = nc.dram_tensor("wsc", [F, 6 * H], f32, kind="ExternalInput").ap()
    wha16 = nc.dram_tensor("wha16", [R, H * HC], f16, kind="ExternalOutput").ap()
    ev = nc.dram_tensor("ev", [R, 6 * H], f32, kind="ExternalOutput").ap()

    with tile.TileContext(nc) as tc:
        with (
            tc.tile_pool(name="sb", bufs=1) as sb,
            tc.tile_pool(name="ps", bufs=4, space="PSUM") as ps,
            tc.tile_pool(name="ob", bufs=4) as ob,
        ):
            # coalesced resident loads (few big DMAs; mm-critical ones first)
            x16g = sb.tile([128, 4, R], f16, tag="x16g")
            nc.sync.dma_start(out=x16g, in_=xT16.rearrange("(g p) r -> p g r", p=128))
            ws16g = sb.tile([128, 4, F], f16, tag="ws16g")
            nc.sync.dma_start(out=ws16g, in_=wsa16.rearrange("(g p) c -> p g c", p=128))
            xg = sb.tile([128, 4, R], f32, tag="xg")
            nc.sync.dma_start(out=xg, in_=xT.rearrange("(g p) r -> p g r", p=128))
            wscg = sb.tile([128, 4, 6 * H], f32, tag="wscg")
            nc.sync.dma_start(out=wscg, in_=wsc.rearrange("(g p) c -> p g c", p=128))

            evt = ob.tile([128, 4, 6 * H], f32, tag="evt", name="evt")
            for nb in range(IC):
                whp = ps.tile([128, 8, NH], f32, tag="whp")
                svp = ps.tile([128, 6 * H], f32, tag="svp")
                for fc in range(4):
                    nc.tensor.matmul(
                        whp, x16g[:, fc, nb * 128:(nb + 1) * 128], ws16g[:, fc, :],
                        start=(fc == 0), stop=(fc == 3),
                    )
                for fc in range(4):
                    nc.tensor.matmul(
                        svp, xg[:, fc, nb * 128:(nb + 1) * 128], wscg[:, fc, :],
                        start=(fc == 0), stop=(fc == 3),
                    )
                # [128, 8, 65] staging: ones col preset, one strided copy
                wt = ob.tile([128, 8, HC], f16, tag="wt")
                nc.vector.memset(wt[:, :, NH:HC], 1.0 / 64.0)
                nc.vector.tensor_copy(wt[:, :, 0:NH], whp)
                nc.sync.dma_start(out=wha16[nb * 128:(nb + 1) * 128, :], in_=wt)

                nc.scalar.activation(evt[:, nb, 0:4 * H], svp[:, 0:4 * H], AF.Exp)
                nc.vector.tensor_copy(evt[:, nb, 4 * H:6 * H], svp[:, 4 * H:6 * H])
            nc.sync.dma_start(
                out=ev.rearrange("(g p) c -> p g c", p=128), in_=evt
            )
    nc.compile()
    return nc


# ---------------------------------------------------------------- k1
K1_A = 52  # blocks routed via ACT (PE additive mask + Prelu/Exp); h5 + part h6
K1_Q = 52  # blocks routed via GPSIMD/Pool; h7 + rest of h6 + rest of h4
# remaining 256-A-Q blocks on DVE: h0..h3 + part of h4
K1_D = 256 - K1_A - K1_Q


def _quadify(jbs):
    """Split a jb list into group-aligned runs of 4 plus single leftovers."""
    out, i = [], 0
    jbs = list(jbs)
    while i < len(jbs):
        j = jbs[i]
        if j % 4 == 0 and jbs[i:i + 4] == [j, j + 1, j + 2, j + 3]:
            out.append(jbs[i:i + 4])
            i += 4
        else:
            out.append([j])
            i += 1
    return out


def _k1_assignment():
    d = K1_D
    assert 128 <= d <= 160 and 32 <= K1_A <= 64
    # DVE works head PAIRS interleaved by jb-quad so each arriving DMA group
    # offers ~4.5us of DVE work (single-head order starves on the loads)
    dve = []
    for h0 in (0, 2):
        for jq in range(JB // 4):
            for h in (h0, h0 + 1):
                dve.append((h, list(range(4 * jq, 4 * jq + 4))))
    # pool takes h7 + the tail of h4; DVE finishes h4's head and h6's
    # scaled tail so pool never touches the straddle head
    n4d = d - 128
    n6d = JB - (K1_A - 32)
    dve += [(4, q) for q in _quadify(range(n4d))]
    dve += [(6, q) for q in _quadify(range(K1_A - 32, JB))]
    act = [(5, [jb]) for jb in range(JB)] + [(6, [jb]) for jb in range(K1_A - 32)]
    pool = (
        [(7, q) for q in _quadify(range(JB))]
        + [(4, q) for q in _quadify(range(n4d, JB))]
    )
    streams = {"dve": dve, "act": act, "pool": pool}
    act_heads = {h for (h, _) in act}
    scaled_heads = {h for (h, _) in dve} | {h for (h, _) in pool}
    return streams, act_heads, scaled_heads


def _unit_cost(e, n):
    if e == "dve":
        return n * 239.0 + (n * 512 * 0.5208 + 156.0)
    if e == "pool":
        return n * 868.0 + (n * 512 * 1.984 + 160.0)
    return 1662.0 * n


def _build_k1():
    """Per-core: 8-head attention for this core's R query rows + Who projection.

    Masked-softmax blocks routed across three engines in proportion to their
    cost-model throughput:
      DVE/Pool route (row-scaled by exp(-s1[i])):
        w = max(f1b*F2c, E2c) (tensor_scalar), p = w*adjT (tensor_tensor over
        a whole group of 4 key blocks at once - amortizes the per-op ramp)
      ACT route (unscaled): e' = (s1[i]-BIG) + BIG*adjT on PE,
        p = exp(lrelu(e'+s2[j])) via Prelu+Exp on ACT.
    outT[h] = sum_jb whaT_h @ p accumulates transposed [65, R]; the ones col
    (valued 1/64) gives rowsum/64 in row 64.  Finalize stays transposed:
    rowsum row -> f16 -> PE broadcast -> tensor divide -> ELU, leaving
    hcatT[h] [64, R] f16 which feeds Who matmuls as 64-partition stationaries
    (no transposes anywhere).  A head whose blocks span the ACT route and a
    scaled route keeps two psum accumulators, merged with exp(-s1) in
    finalize.  All broadcast rows (f1b, s1-BIG, exp(-s1)) are replicated on
    the host from k0's ev output - layout-only prep, no host transcendentals.

    in:  wha  [N, H*HC] f16 (full, from k0; ones cols are 1/64)
         evf  [N, 6H]  f32 (full)
         f1ball [128, H, R] f16, s1rball [1, H, R] f16, e1ball [65, R] f16
         adjt [N, R]   f16 (adj[my rows, :]^T, host prep)
         woa [F, F] f16, wosv16 [F, 2] f16
    out: whoa16 [R, F+1] f16 (hcat@Wo + ones col), svo [R, 3] f32
         (s1o, s2o, exp(-0.8 s1o))
    """
    nc = bacc.Bacc("TRN2", target_bir_lowering=False, debug=False, num_devices=M)
    wha = nc.dram_tensor("wha", [N, H * HC], f16, kind="ExternalInput").ap()
    evf = nc.dram_tensor("evf", [N, 6 * H], f32, kind="ExternalInput").ap()
    f1ball = nc.dram_tensor("f1ball", [128, H, R], f16, kind="ExternalInput").ap()
    s1rball = nc.dram_tensor("s1rball", [1, H, R], f16, kind="ExternalInput").ap()
    e1ball = nc.dram_tensor("e1ball", [65, R], f16, kind="ExternalInput").ap()
    adjt = nc.dram_tensor("adjt", [N, R], f16, kind="ExternalInput").ap()
    woa = nc.dram_tensor("woa", [F, F], f16, kind="ExternalInput").ap()
    wosv16 = nc.dram_tensor("wosv16", [F, 2], f16, kind="ExternalInput").ap()
    whoa16 = nc.dram_tensor("whoa16", [R, F + 1], f16, kind="ExternalOutput").ap()
    svo = nc.dram_tensor("svo", [R, 3], f32, kind="ExternalOutput").ap()

    streams, act_heads, scaled_heads = _k1_assignment()
    straddle_heads = act_heads & scaled_heads
    assert len(straddle_heads) <= 1
    expected = {}
    for eng, units in streams.items():
        for (h, jbs) in units:
            key = (h, "act" if eng == "act" else "sc")
            expected[key] = expected.get(key, 0) + len(jbs)

    with tile.TileContext(nc) as tc:
        with (
            tc.tile_pool(name="sb", bufs=1) as sb,
            tc.tile_pool(name="work", bufs=6) as work,
            tc.tile_pool(name="pw", bufs=14) as pw,
        ):
            ident = sb.tile([128, 128], f32, tag="ident")
            make_identity(nc, ident)
            ones16 = sb.tile([1, 128], f16, tag="ones16")
            nc.vector.memset(ones16, 1.0)
            ones65 = sb.tile([65, 64], f16, tag="ones65")
            nc.vector.memset(ones65, 1.0)
            bigi = sb.tile([128, 128], f16, tag="bigi")
            nc.vector.tensor_scalar(bigi, ident, BIG, None, op0=OP.mult)

            # --- resident loads: SP queue carries evb+adjt, ACT queue carries
            # the broadcast tables + wha + Wo so first groups land early ---
            GB = 4                      # jb blocks per DMA group
            NG = JB // GB               # 8 groups
            adjt_g = adjt.rearrange("(g b p) r -> g p b r", b=GB, p=128)
            wha_g = wha.rearrange("(g b p) c -> g p b c", b=GB, p=128)
            evb = sb.tile([128, JB, 6 * H], f32, tag="evb")
            nc.sync.dma_start(
                out=evb, in_=evf.rearrange("(b p) c -> p b c", p=128)
            )
            f1bt = sb.tile([128, H, R], f16, tag="f1bt")
            nc.scalar.dma_start(out=f1bt, in_=f1ball)
            s1rt = sb.tile([1, H, R], f16, tag="s1rt")
            nc.scalar.dma_start(out=s1rt, in_=s1rball)
            e1bt = sb.tile([65, R], f16, tag="e1bt")
            nc.scalar.dma_start(out=e1bt, in_=e1ball)
            woag = sb.tile([64, H, F], f16, tag="woag")
            nc.scalar.dma_start(out=woag, in_=woa.rearrange("(h p) c -> p h c", p=64))
            wosvg = sb.tile([64, H, 2], f16, tag="wosvg")
            nc.scalar.dma_start(out=wosvg, in_=wosv16.rearrange("(h p) c -> p h c", p=64))
            adjtb, whab = [], []
            for g in range(NG):
                t = sb.tile([128, GB, R], f16, tag=f"adjtb{g}", name=f"adjtb{g}")
                nc.sync.dma_start(out=t, in_=adjt_g[g])
                adjtb.append(t)
                t = sb.tile([128, GB, H * HC], f16, tag=f"whab{g}", name=f"whab{g}")
                nc.sync.dma_start(out=t, in_=wha_g[g])
                whab.append(t)
            adjts = [adjtb[jb // GB][:, jb % GB, :] for jb in range(JB)]
            whas = [whab[jb // GB][:, jb % GB, :] for jb in range(JB)]
            evs = [evb[:, jb, :] for jb in range(JB)]
            f1bs = {h: f1bt[:, h, :] for h in range(H)}
            s1rbs = {h: s1rt[:, h, :] for h in range(H)}

            hcTs = [
                sb.tile([64, R], f16, tag=f"hcT{h}", name=f"hcT{h}")
                for h in range(H)
            ]
            rs16v = sb.tile([65, R], f16, tag="rs16v")
            rcp16 = sb.tile([65, R], f16, tag="rcp16")

            with (
                tc.tile_pool(name="ap", bufs=1, space="PSUM") as accp,
                tc.tile_pool(name="fz", bufs=1, space="PSUM") as fzp,
            ):
                outts = {}
                counts = {}
                pending = {"dve": [], "act": [], "pool": []}
                PEND_DEPTH = {"dve": 1, "act": 1, "pool": 1}

                # static psum bank plan: heads sharing a tag have disjoint
                # accumulation lifetimes, so no ring wait can convoy PE
                BANK_TAG = {
                    (0, "sc"): "tA", (3, "sc"): "tA",
                    (1, "sc"): "tB", (6, "sc"): "tB",
                    (2, "sc"): "tC", (4, "sc"): "tC",
                    (7, "sc"): "tD",
                    (5, "act"): "tE", (6, "act"): "tF",
                    (5, "sc"): "tD", (4, "act"): "tF", (7, "act"): "tE",
                }

                def get_outt(h, cls):
                    key = (h, cls)
                    if key not in outts:
                        outts[key] = accp.tile(
                            [HC, R], f32, tag=BANK_TAG[key],
                            name=f"outt_{cls}{h}", bufs=1,
                        )
                        counts[key] = 0
                    return outts[key]

                def emit_vmm(h, cls, jb, p):
                    outt = get_outt(h, cls)
                    counts[(h, cls)] += 1
                    nc.tensor.matmul(
                        outt, whas[jb][:, h * HC:(h + 1) * HC], p,
                        start=(counts[(h, cls)] == 1),
                        stop=(counts[(h, cls)] == expected[(h, cls)]),
                    )

                def flush(e, all_=True):
                    while pending[e] and (all_ or len(pending[e]) > PEND_DEPTH[e]):
                        for args in pending[e].pop(0):
                            emit_vmm(*args)

                dve_units_left = [len(streams["dve"])]

                def emit_scaled(ename, h, jbs):
                    # n tensor_scalars (one per block: the per-partition
                    # scalars differ per key block) + ONE wide mask multiply
                    # over the whole aligned run; value matmuls deferred one
                    # unit so PE never waits on an in-flight p.  Late DVE
                    # units hand the tail columns of the mask multiply to
                    # Pool, which has gone idle by then.
                    eng = {"dve": nc.vector, "pool": nc.gpsimd}[ename]
                    n = len(jbs)
                    g, b0 = jbs[0] // GB, jbs[0] % GB
                    w = pw.tile([128, n, R], f16, tag=f"w{n}_{ename}", bufs=2 if n == 4 else 4)
                    for k, jb in enumerate(jbs):
                        eng.tensor_scalar(
                            w[:, k, :], f1bs[h], evs[jb][:, H + h:H + h + 1],
                            evs[jb][:, h:h + 1], op0=OP.mult, op1=OP.max,
                        )
                    p = pw.tile([128, n, R], f16, tag=f"p{n}_{ename}", bufs=3 if n == 4 else 4)
                    split = 0
                    if ename == "dve":
                        dve_units_left[0] -= 1
                        if dve_units_left[0] < 15:
                            split = 320
                    if split:
                        eng.tensor_tensor(
                            p[:, :, 0:split], w[:, :, 0:split],
                            adjtb[g][:, b0:b0 + n, 0:split], op=OP.mult,
                        )
                        nc.gpsimd.tensor_tensor(
                            p[:, :, split:R], w[:, :, split:R],
                            adjtb[g][:, b0:b0 + n, split:R], op=OP.mult,
                        )
                    else:
                        eng.tensor_tensor(
                            p, w, adjtb[g][:, b0:b0 + n, :], op=OP.mult
                        )
                    pending[ename].append(
                        [(h, "sc", jb, p[:, k, :]) for k, jb in enumerate(jbs)]
                    )
                    flush(ename, all_=False)

                def emit_act(h, jbs):
                    (jb,) = jbs
                    eps = fzp.tile([128, R], f32, tag="eps", bufs=1)
                    nc.tensor.matmul(eps, ones16, s1rbs[h], start=True, stop=False)
                    nc.tensor.matmul(eps, bigi, adjts[jb], start=False, stop=True)
                    m = pw.tile([128, R], f16, tag="m", bufs=4)
                    nc.scalar.activation(
                        m, eps, AF.Prelu, alpha=ALPHA,
                        bias=evb[:, jb, 5 * H + h:5 * H + h + 1],
                    )
                    p = pw.tile([128, R], f16, tag="p2", bufs=5)
                    nc.scalar.activation(p, m, AF.Exp)
                    pending["act"].append([(h, "act", jb, p)])
                    flush("act", all_=False)

                fin_pending = []

                def finalize_a(h, fin_eng):
                    # transposed finalize.  HW allows only ONE psum input per
                    # vector op, so the recip broadcast is copied to SBUF
                    # (on the finishing stream's engine) before the multiply.
                    if h in straddle_heads:
                        osc, oac = outts[(h, "sc")], outts[(h, "act")]
                        cmb = pw.tile([65, R], f16, tag="cmb", bufs=1)
                        nc.vector.tensor_tensor(cmb, oac, e1bt, op=OP.mult)
                        nc.vector.tensor_tensor(
                            rs16v[64:65, :], osc[64:65, :], cmb[64:65, :], op=OP.add
                        )
                        tq = pw.tile([64, R], f16, tag="tq", bufs=2)
                        nc.vector.scalar_tensor_tensor(
                            tq, osc[0:64, :], 0.0, cmb[0:64, :],
                            op0=OP.add, op1=OP.add,
                        )
                        num = tq
                        with nc.allow_low_precision(reason="softmax recip row"):
                            nc.vector.reciprocal(rcp16[64:65, :], rs16v[64:65, :])
                    else:
                        osc = outts.get((h, "sc"))
                        if osc is None:
                            osc = outts[(h, "act")]
                        num = osc[0:64, :]
                        with nc.allow_low_precision(reason="softmax recip row"):
                            nc.vector.reciprocal(rcp16[64:65, :], osc[64:65, :])
                    rb = fzp.tile([64, R], f32, tag="rb")
                    nc.tensor.matmul(
                        rb, ones65[64:65, :], rcp16[64:65, :], start=True, stop=True
                    )
                    rb16 = pw.tile([64, R], f16, tag="rb16", bufs=2)
                    nc.vector.tensor_copy(rb16, rb)
                    t = pw.tile([64, R], f16, tag="t", bufs=2)
                    nc.vector.tensor_tensor(t, num, rb16, op=OP.mult)
                    m0 = pw.tile([64, R], f16, tag="m0", bufs=2)
                    nc.vector.tensor_scalar(
                        m0, t, 1.0 / 64.0, 0.0, op0=OP.mult, op1=OP.min
                    )
                    ex = pw.tile([64, R], f16, tag="ex", bufs=3)
                    nc.scalar.activation(ex, m0, AF.Exp)
                    rl2 = pw.tile([64, R], f16, tag="rl2", bufs=3)
                    nc.vector.tensor_scalar(
                        rl2, t, 1.0 / 64.0, 0.0, op0=OP.mult, op1=OP.max
                    )
                    fin_pending.append([0, h, ex, rl2])

                def fin_tick(force=False):
                    for item in list(fin_pending):
                        item[0] += 1
                        if force or item[0] > 2:
                            _, h, ex, rl2 = item
                            nc.vector.tensor_tensor(hcTs[h], ex, rl2, op=OP.add)
                            nc.vector.tensor_scalar(
                                hcTs[h], hcTs[h], -1.0, None, op0=OP.add
                            )
                            fin_pending.remove(item)

                # --- merged emission by virtual engine clocks ---
                # finalize is deferred 2 units behind the stream that emitted
                # the head's last block: the engine executes behind emission,
                # and an early inline finalize stalls DVE's in-order stream
                fin_a_pending = {"dve": [], "act": [], "pool": []}

                FIN_DEPTH = {"dve": 2, "act": 2, "pool": 2}

                def fin_a_tick(e, force=False):
                    for item in list(fin_a_pending[e]):
                        item[0] += 1
                        if force or item[0] > FIN_DEPTH[e]:
                            finalize_a(item[1], e)
                            clocks["dve"] += 2300.0
                            clocks["act"] += 831.0
                            fin_a_pending[e].remove(item)

                clocks = {"dve": 0.0, "act": 0.0, "pool": 0.0}
                pos = {e: 0 for e in streams}
                head_done = {h: 0 for h in range(H)}
                remaining = sum(len(u) for u in streams.values())
                while remaining:
                    cand = []
                    for e in streams:
                        if pos[e] < len(streams[e]):
                            n = len(streams[e][pos[e]][1])
                            cand.append((clocks[e] + _unit_cost(e, n), e))
                    _, e = min(cand)
                    h, jbs = streams[e][pos[e]]
                    pos[e] += 1
                    remaining -= 1
                    clocks[e] += _unit_cost(e, len(jbs))
                    if e == "act":
                        emit_act(h, jbs)
                    else:
                        emit_scaled(e, h, jbs)
                    head_done[h] += len(jbs)
                    fin_tick()
                    fin_a_tick(e)
                    if pos[e] == len(streams[e]):
                        for e2 in ("dve", "act", "pool"):
                            flush(e2)
                        fin_a_tick(e, force=True)
                    if head_done[h] == JB:
                        for e2 in ("dve", "act", "pool"):
                            flush(e2)
                        fin_a_pending[e].append([0, h])
                for e2 in ("dve", "act", "pool"):
                    flush(e2)
                for e in fin_a_pending:
                    fin_a_tick(e, force=True)
                fin_tick(force=True)

            # --- Who = hcat @ [Wo | w1 | w2] for my rows (64-part stationaries)
            with tc.tile_pool(name="fp2", bufs=2, space="PSUM") as fp2:
                for ic in range(IC):
                    wop = fp2.tile([128, F], f32, tag="wop")
                    svp = fp2.tile([128, 2], f32, tag="svp2")
                    for h in range(H):
                        nc.tensor.matmul(
                            wop, hcTs[h][:, ic * 128:(ic + 1) * 128], woag[:, h, :],
                            start=(h == 0), stop=(h == H - 1),
                        )
                    for h in range(H):
                        nc.tensor.matmul(
                            svp, hcTs[h][:, ic * 128:(ic + 1) * 128], wosvg[:, h, :],
                            start=(h == 0), stop=(h == H - 1),
                        )
                    wt = work.tile([128, F + 1], f16, tag="wt")
                    nc.scalar.activation(wt[:, 0:F], wop, AF.Copy)
                    nc.vector.memset(wt[:, F:F + 1], 1.0)
                    nc.sync.dma_start(
                        out=whoa16[ic * 128:(ic + 1) * 128, :], in_=wt
                    )
                    st = work.tile([128, 3], f32, tag="st")
                    nc.vector.tensor_copy(st[:, 0:2], svp)
                    nc.scalar.activation(st[:, 2:3], svp[:, 0:1], AF.Exp, scale=-0.8)
                    nc.sync.dma_start(out=svo[ic * 128:(ic + 1) * 128, :], in_=st)
    nc.compile()
    return nc


# ---------------------------------------------------------------- k2
def _build_k2():
    """Per-core: output-layer attention for this core's R rows, final ELU.

    in:  whoa [N, F+1] f16, svof [N, 2] f32, svomy [R, 2] f32, adjt [N, R] f16
    out: out [R, F] f32
    """
    nc = bacc.Bacc("TRN2", target_bir_lowering=False, debug=False, num_devices=M)
    whoa = nc.dram_tensor("whoa", [N, F + 1], f16, kind="ExternalInput").ap()
    svof = nc.dram_tensor("svof", [N, 2], f32, kind="ExternalInput").ap()
    svomy = nc.dram_tensor("svomy", [R, 2], f32, kind="ExternalInput").ap()
    adjt = nc.dram_tensor("adjt", [N, R], f16, kind="ExternalInput").ap()
    out = nc.dram_tensor("out", [R, F], f16, kind="ExternalOutput").ap()

    with tile.TileContext(nc) as tc:
        with (
            tc.tile_pool(name="sb", bufs=1) as sb,
            tc.tile_pool(name="work", bufs=10) as work,
        ):
            # --- prep first (small DMAs ahead of the big resident loads) ---
            ident = sb.tile([128, 128], f32, tag="ident")
            make_identity(nc, ident)
            ones1 = sb.tile([1, 128], f32, tag="ones1")
            nc.vector.memset(ones1, 1.0)

            # blocked s2 [128, 32]: col b = s2o[b*128 + p]
            s2blk = sb.tile([128, JB], f32, tag="s2blk")
            nc.sync.dma_start(
                out=s2blk, in_=svof.rearrange("(b p) c -> p b c", p=128)[:, :, 1]
            )
            e2c = sb.tile([128, JB], f32, tag="e2c")
            f2c = sb.tile([128, JB], f32, tag="f2c")
            f1bo = sb.tile([128, R], f16, tag="f1bo")

            with tc.tile_pool(name="pp", bufs=2, space="PSUM") as pp:
                # global max of s2o -> stability shift: bias = 9 - max(s2o)
                mx1 = work.tile([128, 1], f32, tag="mx1")
                nc.vector.tensor_reduce(mx1, s2blk, axis=mybir.AxisListType.X, op=OP.max)
                mxp = pp.tile([1, 128], f32, tag="mxp")
                nc.tensor.transpose(mxp, mx1, ident)
                mxs = work.tile([1, 128], f32, tag="mxs")
                nc.vector.tensor_copy(mxs, mxp)
                mx2 = work.tile([1, 1], f32, tag="mx2")
                nc.vector.tensor_reduce(mx2, mxs, axis=mybir.AxisListType.X, op=OP.max)
                bias1 = work.tile([1, 1], f32, tag="bias1")
                nc.vector.tensor_scalar(
                    bias1, mx2, -1.0, 9.0, op0=OP.mult, op1=OP.add
                )
                biasb = sb.tile([128, 1], f32, tag="biasb")
                bp = pp.tile([128, 1], f32, tag="bp")
                nc.tensor.matmul(bp, ones1, bias1, start=True, stop=True)
                nc.vector.tensor_copy(biasb, bp)
                nc.scalar.activation(e2c, s2blk, AF.Exp, bias=biasb)
                nc.scalar.activation(f2c, s2blk, AF.Exp, bias=biasb, scale=0.2)

                # F1' broadcast tile from my s1o
                s1row = sb.tile([1, R], f32, tag="s1row")
                nc.sync.dma_start(
                    out=s1row, in_=svomy[:, 0:1].rearrange("r one -> one r")
                )
                f1row = work.tile([1, R], f32, tag="f1row")
                nc.scalar.activation(f1row, s1row, AF.Exp, scale=-0.8)
                fbp = pp.tile([128, R], f32, tag="fbp")
                nc.tensor.matmul(fbp, ones1, f1row, start=True, stop=True)
                nc.scalar.activation(f1bo, fbp, AF.Copy)

            # --- resident loads, coalesced grouped 3D-AP DMAs ---
            GB = 8
            NG = JB // GB
            adjt_g = adjt.rearrange("(g b p) r -> g p b r", b=GB, p=128)
            whoa_g = whoa.rearrange("(g b p) c -> g p b c", b=GB, p=128)
            adjtb, whob = [], []
            for g in range(NG):
                t = sb.tile([128, GB, R], f16, tag=f"adjtb{g}", name=f"adjtb{g}")
                nc.sync.dma_start(out=t, in_=adjt_g[g])
                adjtb.append(t)
                t = sb.tile([128, GB, F + 1], f16, tag=f"whob{g}", name=f"whob{g}")
                nc.sync.dma_start(out=t, in_=whoa_g[g])
                whob.append(t)
            adjts = [adjtb[jb // GB][:, jb % GB, :] for jb in range(JB)]
            whos = [whob[jb // GB][:, jb % GB, :] for jb in range(JB)]

            # --- main loop ---
            with tc.tile_pool(name="ap", bufs=1, space="PSUM") as accp:
                accs = [accp.tile([128, F], f32, tag=f"acc{ic}", name=f"acc{ic}") for ic in range(IC)]
                rss = [accp.tile([128, 1], f32, tag=f"rs{ic}", name=f"rs{ic}") for ic in range(IC)]
                SP = 384  # DVE takes [0:SP), GPSIMD [SP:R) - parallel halves
                for jb in range(JB):
                    w = work.tile([128, R], f16, tag="w")
                    p = work.tile([128, R], f16, tag="p")
                    nc.vector.tensor_scalar(
                        w[:, 0:SP], f1bo[:, 0:SP], f2c[:, jb:jb + 1],
                        e2c[:, jb:jb + 1], op0=OP.mult, op1=OP.max,
                    )
                    nc.vector.tensor_tensor(
                        p[:, 0:SP], w[:, 0:SP], adjts[jb][:, 0:SP], op=OP.mult
                    )
                    nc.gpsimd.tensor_scalar(
                        w[:, SP:R], f1bo[:, SP:R], f2c[:, jb:jb + 1],
                        e2c[:, jb:jb + 1], op0=OP.mult, op1=OP.max,
                    )
                    nc.gpsimd.tensor_tensor(
                        p[:, SP:R], w[:, SP:R], adjts[jb][:, SP:R], op=OP.mult
                    )
                    for ic in range(IC):
                        nc.tensor.matmul(
                            accs[ic], p[:, ic * 128:(ic + 1) * 128],
                            whos[jb][:, 0:F],
                            start=(jb == 0), stop=(jb == JB - 1),
                        )
                        nc.tensor.matmul(
                            rss[ic], p[:, ic * 128:(ic + 1) * 128],
                            whos[jb][:, F:F + 1],
                            start=(jb == 0), stop=(jb == JB - 1),
                        )
                for ic in range(IC):
                    r = work.tile([128, 1], f32, tag="r")
                    nc.vector.reciprocal(r, rss[ic])
                    ot = work.tile([128, F], f16, tag="ot")
                    nc.scalar.activation(ot, accs[ic], AF.Copy, scale=r)
                    m0 = work.tile([128, F], f16, tag="m0")
                    nc.vector.tensor_scalar(m0, ot, 0.0, None, op0=OP.min)
                    ex = work.tile([128, F], f16, tag="ex")
                    nc.scalar.activation(ex, m0, AF.Exp)
                    rl2 = work.tile([128, F], f16, tag="rl2")
                    nc.vector.tensor_scalar(rl2, ot, 0.0, None, op0=OP.max)
                    res = work.tile([128, F], f16, tag="res")
                    nc.vector.tensor_tensor(res, ex, rl2, op=OP.add)
                    nc.vector.tensor_scalar(res, res, -1.0, None, op0=OP.add)
                    nc.sync.dma_start(out=out[ic * 128:(ic + 1) * 128, :], in_=res)
    nc.compile()
    return nc


def _get(name):
    if name not in _CACHE:
        _CACHE[name] = {"k0": _build_k0, "k1": _build_k1, "k2": _build_k2}[name]()
    return _CACHE[name]


# ---------------------------------------------------------------- host
def kernel(x, left, adj, Ws, a1, a2, Wo, ao1, ao2):
    x = np.asarray(x, np.float32)
    adj = np.asarray(adj, np.float32)
    Ws = np.asarray(Ws, np.float32)
    a1 = np.asarray(a1, np.float32)
    a2 = np.asarray(a2, np.float32)
    Wo = np.asarray(Wo, np.float32)
    ao1 = np.asarray(ao1, np.float32)
    ao2 = np.asarray(ao2, np.float32)

    # host-side layout prep (no significant FLOPs)
    ws_all = np.ascontiguousarray(Ws.transpose(1, 0, 2).reshape(F, F))
    ws1 = np.einsum("hkf,hf->kh", Ws, a1)   # [F, H]  tiny matvecs
    ws2 = np.einsum("hkf,hf->kh", Ws, a2)
    wsc = np.ascontiguousarray(
        np.concatenate([ws2, 0.2 * ws2, -0.8 * ws1, -ws1, ws1, ws2], axis=1),
        dtype=np.float32,
    )
    wsa16 = ws_all.astype(np.float16)
    woa = np.ascontiguousarray(Wo).astype(np.float16)
    wosv16 = np.ascontiguousarray(
        np.stack([Wo @ ao1, Wo @ ao2], axis=1), dtype=np.float16
    )
    adj16 = adj.astype(np.float16)  # exact: adj is a 0/1 mask
    adjt_c = [
        np.ascontiguousarray(adj16[c * R:(c + 1) * R].T) for c in range(M)
    ]
    xt_c = [np.ascontiguousarray(x[c * R:(c + 1) * R].T) for c in range(M)]

    cores = list(range(M))

    k0 = _get("k0")
    res0 = _run(
        k0,
        [
            {
                "xT": xt_c[c],
                "xT16": xt_c[c].astype(np.float16),
                "wsa16": wsa16,
                "wsc": wsc,
            }
            for c in cores
        ],
        cores,
    )
    wha = np.concatenate([res0.results[c]["wha16"] for c in cores], axis=0)
    evf = np.concatenate([res0.results[c]["ev"] for c in cores], axis=0)

    # broadcast tables for k1, replicated (layout only) from k0's ev output
    streams, act_heads, scaled_heads = _k1_assignment()
    straddle = sorted(act_heads & scaled_heads)
    in1 = []
    for c in cores:
        evmy = evf[c * R:(c + 1) * R]
        f1ball = np.ascontiguousarray(np.broadcast_to(
            evmy[:, 2 * H:3 * H].T[None, :, :], (128, H, R)
        ).astype(np.float16))
        s1rball = np.ascontiguousarray(
            (evmy[:, 4 * H:5 * H].T[None, :, :] - BIG).astype(np.float16)
        )
        if straddle:
            e1ball = np.ascontiguousarray(np.broadcast_to(
                evmy[:, 3 * H + straddle[0]][None, :], (65, R)
            ).astype(np.float16))
        else:
            e1ball = np.zeros((65, R), np.float16)
        in1.append(
            {
                "wha": wha,
                "evf": evf,
                "f1ball": f1ball,
                "s1rball": s1rball,
                "e1ball": e1ball,
                "adjt": adjt_c[c],
                "woa": woa,
                "wosv16": wosv16,
            }
        )
    k1 = _get("k1")
    res1 = _run(k1, in1, cores)
    whoa = np.concatenate([res1.results[c]["whoa16"] for c in cores], axis=0)
    svof = np.concatenate([res1.results[c]["svo"] for c in cores], axis=0)

    svof2 = np.ascontiguousarray(svof[:, 0:2])
    k2 = _get("k2")
    res2 = _run(
        k2,
        [
            {
                "whoa": whoa,
                "svof": svof2,
                "svomy": svof2[c * R:(c + 1) * R],
                "adjt": adjt_c[c],
            }
            for c in cores
        ],
        cores,
    )
    return np.concatenate(
        [res2.results[c]["out"] for c in cores], axis=0
    ).astype(np.float32)


# revision 61
# speedup vs baseline: 1.0436x; 1.0034x over previous
"""GAT (graph attention network) forward pass on 8 Trainium2 NeuronCores.

Problem: nn_GAT - N=4096 nodes, F=512 features, H=8 heads, 1% dense adjacency.
    heads:  Wh = x @ Ws[h]; e = lrelu(s1[i]+s2[j]); att = masked softmax; elu(att @ Wh)
    out layer: same attention structure on hcat @ Wo, then elu.

Strategy (row-sharded across 8 cores, 3 launches):
  k0: each core computes Wh (all heads, fused matmul) + score vectors for its
      512 nodes; host gathers.
  k1: each core runs 8-head masked-softmax attention for its 512 query rows
      (key insight: exp(lrelu(e)) = max(exp(e), exp(0.2e)) and exp(e) factors
      rank-1 as exp(s1)[i]*exp(s2)[j], so the NxN tiles need NO transcendentals
      and NO PSUM e-matrix - just tensor_scalar/tensor_tensor ops in fp16.
      Softmax is invariant to per-query-row scaling, so scaling row i by
      exp(-s1[i]) turns the exp(e) branch into a per-partition scalar:
        u'[j,i] = exp(s2[j])          (tensor_scalar max)
        v'[j,i] = exp(-0.8*s1[i]) * exp(0.2*s2[j])   (tensor_scalar mult)
        p = max(u', v') * adjT        (mask multiply, fp16)
      The softmax denominator comes free as a ones-column in the value matrix.
      Blocks are routed across DVE, ACT (additive-mask PE+activation form) and
      GPSIMD in proportion to their simulated throughput.
      Also computes hcat @ Wo (+ output-layer score vectors) for its rows.
  k2: output-layer attention for the core's 512 rows; final ELU.

adj is passed from host as a pre-transposed fp16 (exact for a 0/1 mask) slice
per core; x is passed pre-transposed fp32 (pure layout prep, no FLOPs).
"""

import sys

for _p in ("/opt/trn_rl_repo",):
    if _p not in sys.path:
        sys.path.insert(0, _p)

import numpy as np

import concourse.bass as bass
import concourse.tile as tile
from concourse import bacc, mybir
from concourse.bass_utils import run_bass_kernel_spmd
from concourse.masks import make_identity

N, F, H, NH = 4096, 512, 8, 64
M = 8            # cores
R = N // M       # 512 query rows per core
JB = N // 128    # 32 key blocks
IC = R // 128    # 4 query-row chunks per core
HC = NH + 1      # 65 value cols per head (64 + ones col for row sums)
ALPHA = 0.2     # leaky relu slope
BIG = 200.0     # additive mask; 0.2*BIG=40 so masked exp underflows to 0
f32 = mybir.dt.float32
f16 = mybir.dt.float16
OP = mybir.AluOpType
AF = mybir.ActivationFunctionType

_CACHE = {}


def _run(nc, in_maps, core_ids, tries=3):
    """run_bass_kernel_spmd with retry: the axon-tunneled devices
    occasionally report NRT_EXEC_UNIT_UNRECOVERABLE transiently."""
    import time as _time

    for attempt in range(tries):
        try:
            return run_bass_kernel_spmd(nc, in_maps, core_ids=core_ids)
        except Exception:
            if attempt == tries - 1:
                raise
            _time.sleep(5.0)


# ---------------------------------------------------------------- k0
def _build_k0():
    """Per-core: Wh projection for this core's R nodes, all heads fused.

    in:  xT    [F, R] f32 / xT16 [F, R] f16 (this core's x rows, transposed)
         wsa16 [F, F] f16  (Ws stacked, head-major cols)
         wsc   [F, 6H] f32 (score cols: [ws2|.2ws2|-.8ws1|-ws1|ws1|ws2])
    out: wha16 [R, H*HC] f16 (per head: 64 value cols + ones col)
         ev    [R, 6H] f32 (exp(s2)|exp(.2s2)|exp(-.8s1)|exp(-s1)|s1|s2)
    """
    nc = bacc.Bacc("TRN2", target_bir_lowering=False, debug=False, num_devices=M)
    xT = nc.dram_tensor("xT", [F, R], f32, kind="ExternalInput").ap()
    xT16 = nc.dram_tensor("xT16", [F, R], f16, kind="ExternalInput").ap()
    wsa16 = nc.dram_tensor("wsa16", [F, F], f16, kind="ExternalInput").ap()
    wsc = nc.dram_tensor("wsc", [F, 6 * H], f32, kind="ExternalInput").ap()
    wha16 = nc.dram_tensor("wha16", [R, H * HC], f16, kind="ExternalOutput").ap()
    ev = nc.dram_tensor("ev", [R, 6 * H], f32, kind="ExternalOutput").ap()

    with tile.TileContext(nc) as tc:
        with (
            tc.tile_pool(name="sb", bufs=1) as sb,
            tc.tile_pool(name="ps", bufs=4, space="PSUM") as ps,
            tc.tile_pool(name="ob", bufs=4) as ob,
        ):
            # coalesced resident loads (few big DMAs; mm-critical ones first)
            x16g = sb.tile([128, 4, R], f16, tag="x16g")
            nc.sync.dma_start(out=x16g, in_=xT16.rearrange("(g p) r -> p g r", p=128))
            ws16g = sb.tile([128, 4, F], f16, tag="ws16g")
            nc.sync.dma_start(out=ws16g, in_=wsa16.rearrange("(g p) c -> p g c", p=128))
            xg = sb.tile([128, 4, R], f32, tag="xg")
            nc.sync.dma_start(out=xg, in_=xT.rearrange("(g p) r -> p g r", p=128))
            wscg = sb.tile([128, 4, 6 * H], f32, tag="wscg")
            nc.sync.dma_start(out=wscg, in_=wsc.rearrange("(g p) c -> p g c", p=128))

            evt = ob.tile([128, 4, 6 * H], f32, tag="evt", name="evt")
            for nb in range(IC):
                whp = ps.tile([128, 8, NH], f32, tag="whp")
                svp = ps.tile([128, 6 * H], f32, tag="svp")
                for fc in range(4):
                    nc.tensor.matmul(
                        whp, x16g[:, fc, nb * 128:(nb + 1) * 128], ws16g[:, fc, :],
                        start=(fc == 0), stop=(fc == 3),
                    )
                for fc in range(4):
                    nc.tensor.matmul(
                        svp, xg[:, fc, nb * 128:(nb + 1) * 128], wscg[:, fc, :],
                        start=(fc == 0), stop=(fc == 3),
                    )
                # [128, 8, 65] staging: ones col preset, one strided copy
                wt = ob.tile([128, 8, HC], f16, tag="wt")
                nc.vector.memset(wt[:, :, NH:HC], 1.0 / 64.0)
                nc.vector.tensor_copy(wt[:, :, 0:NH], whp)
                nc.sync.dma_start(out=wha16[nb * 128:(nb + 1) * 128, :], in_=wt)

                nc.scalar.activation(evt[:, nb, 0:4 * H], svp[:, 0:4 * H], AF.Exp)
                nc.vector.tensor_copy(evt[:, nb, 4 * H:6 * H], svp[:, 4 * H:6 * H])
            nc.sync.dma_start(
                out=ev.rearrange("(g p) c -> p g c", p=128), in_=evt
            )
    nc.compile()
    return nc


# ---------------------------------------------------------------- k1
K1_A = 52  # blocks routed via ACT (PE additive mask + Prelu/Exp); h5 + part h6
K1_Q = 52  # blocks routed via GPSIMD/Pool; h7 + rest of h6 + rest of h4
# remaining 256-A-Q blocks on DVE: h0..h3 + part of h4
K1_D = 256 - K1_A - K1_Q


def _quadify(jbs):
    """Split a jb list into group-aligned runs of 4 plus single leftovers."""
    out, i = [], 0
    jbs = list(jbs)
    while i < len(jbs):
        j = jbs[i]
        if j % 4 == 0 and jbs[i:i + 4] == [j, j + 1, j + 2, j + 3]:
            out.append(jbs[i:i + 4])
            i += 4
        else:
            out.append([j])
            i += 1
    return out


def _k1_assignment():
    d = K1_D
    assert 128 <= d <= 160 and 32 <= K1_A <= 64
    # DVE works head PAIRS interleaved by jb-quad so each arriving DMA group
    # offers ~4.5us of DVE work (single-head order starves on the loads)
    dve = []
    for h0 in (0, 2):
        for jq in range(JB // 4):
            for h in (h0, h0 + 1):
                dve.append((h, list(range(4 * jq, 4 * jq + 4))))
    # pool takes h7 + the tail of h4; DVE finishes h4's head and h6's
    # scaled tail so pool never touches the straddle head
    n4d = d - 128
    n6d = JB - (K1_A - 32)
    dve += [(4, q) for q in _quadify(range(n4d))]
    dve += [(6, q) for q in _quadify(range(K1_A - 32, JB))]
    act = [(5, [jb]) for jb in range(JB)] + [(6, [jb]) for jb in range(K1_A - 32)]
    pool = (
        [(7, q) for q in _quadify(range(JB))]
        + [(4, q) for q in _quadify(range(n4d, JB))]
    )
    streams = {"dve": dve, "act": act, "pool": pool}
    act_heads = {h for (h, _) in act}
    scaled_heads = {h for (h, _) in dve} | {h for (h, _) in pool}
    return streams, act_heads, scaled_heads


def _unit_cost(e, n):
    if e == "dve":
        return n * 239.0 + (n * 512 * 0.5208 + 156.0)
    if e == "pool":
        return n * 868.0 + (n * 512 * 1.984 + 160.0)
    return 1662.0 * n


def _build_k1():
    """Per-core: 8-head attention for this core's R query rows + Who projection.

    Masked-softmax blocks routed across three engines in proportion to their
    cost-model throughput:
      DVE/Pool route (row-scaled by exp(-s1[i])):
        w = max(f1b*F2c, E2c) (tensor_scalar), p = w*adjT (tensor_tensor over
        a whole group of 4 key blocks at once - amortizes the per-op ramp)
      ACT route (unscaled): e' = (s1[i]-BIG) + BIG*adjT on PE,
        p = exp(lrelu(e'+s2[j])) via Prelu+Exp on ACT.
    outT[h] = sum_jb whaT_h @ p accumulates transposed [65, R]; the ones col
    (valued 1/64) gives rowsum/64 in row 64.  Finalize stays transposed:
    rowsum row -> f16 -> PE broadcast -> tensor divide -> ELU, leaving
    hcatT[h] [64, R] f16 which feeds Who matmuls as 64-partition stationaries
    (no transposes anywhere).  A head whose blocks span the ACT route and a
    scaled route keeps two psum accumulators, merged with exp(-s1) in
    finalize.  All broadcast rows (f1b, s1-BIG, exp(-s1)) are replicated on
    the host from k0's ev output - layout-only prep, no host transcendentals.

    in:  wha  [N, H*HC] f16 (full, from k0; ones cols are 1/64)
         evf  [N, 6H]  f32 (full)
         f1ball [128, H, R] f16, s1rball [1, H, R] f16, e1ball [65, R] f16
         adjt [N, R]   f16 (adj[my rows, :]^T, host prep)
         woa [F, F] f16, wosv16 [F, 2] f16
    out: whoa16 [R, F+1] f16 (hcat@Wo + ones col), svo [R, 3] f32
         (s1o, s2o, exp(-0.8 s1o))
    """
    nc = bacc.Bacc("TRN2", target_bir_lowering=False, debug=False, num_devices=M)
    wha = nc.dram_tensor("wha", [N, H * HC], f16, kind="ExternalInput").ap()
    evf = nc.dram_tensor("evf", [N, 6 * H], f32, kind="ExternalInput").ap()
    f1ball = nc.dram_tensor("f1ball", [128, H, R], f16, kind="ExternalInput").ap()
    s1rball = nc.dram_tensor("s1rball", [1, H, R], f16, kind="ExternalInput").ap()
    e1ball = nc.dram_tensor("e1ball", [65, R], f16, kind="ExternalInput").ap()
    adjt = nc.dram_tensor("adjt", [N, R], f16, kind="ExternalInput").ap()
    woa = nc.dram_tensor("woa", [F, F], f16, kind="ExternalInput").ap()
    wosv16 = nc.dram_tensor("wosv16", [F, 2], f16, kind="ExternalInput").ap()
    whoa16 = nc.dram_tensor("whoa16", [R, F + 1], f16, kind="ExternalOutput").ap()
    svo = nc.dram_tensor("svo", [R, 3], f32, kind="ExternalOutput").ap()

    streams, act_heads, scaled_heads = _k1_assignment()
    straddle_heads = act_heads & scaled_heads
    assert len(straddle_heads) <= 1
    expected = {}
    for eng, units in streams.items():
        for (h, jbs) in units:
            key = (h, "act" if eng == "act" else "sc")
            expected[key] = expected.get(key, 0) + len(jbs)

    with tile.TileContext(nc) as tc:
        with (
            tc.tile_pool(name="sb", bufs=1) as sb,
            tc.tile_pool(name="work", bufs=6) as work,
            tc.tile_pool(name="pw", bufs=14) as pw,
        ):
            ident = sb.tile([128, 128], f32, tag="ident")
            make_identity(nc, ident)
            ones16 = sb.tile([1, 128], f16, tag="ones16")
            nc.vector.memset(ones16, 1.0)
            ones65 = sb.tile([65, 64], f16, tag="ones65")
            nc.vector.memset(ones65, 1.0)
            bigi = sb.tile([128, 128], f16, tag="bigi")
            nc.vector.tensor_scalar(bigi, ident, BIG, None, op0=OP.mult)

            # --- resident loads: SP queue carries evb+adjt, ACT queue carries
            # the broadcast tables + wha + Wo so first groups land early ---
            GB = 4                      # jb blocks per DMA group
            NG = JB // GB               # 8 groups
            adjt_g = adjt.rearrange("(g b p) r -> g p b r", b=GB, p=128)
            wha_g = wha.rearrange("(g b p) c -> g p b c", b=GB, p=128)
            evb = sb.tile([128, JB, 6 * H], f32, tag="evb")
            nc.sync.dma_start(
                out=evb, in_=evf.rearrange("(b p) c -> p b c", p=128)
            )
            f1bt = sb.tile([128, H, R], f16, tag="f1bt")
            nc.scalar.dma_start(out=f1bt, in_=f1ball)
            s1rt = sb.tile([1, H, R], f16, tag="s1rt")
            nc.scalar.dma_start(out=s1rt, in_=s1rball)
            e1bt = sb.tile([65, R], f16, tag="e1bt")
            nc.scalar.dma_start(out=e1bt, in_=e1ball)
            woag = sb.tile([64, H, F], f16, tag="woag")
            nc.scalar.dma_start(out=woag, in_=woa.rearrange("(h p) c -> p h c", p=64))
            wosvg = sb.tile([64, H, 2], f16, tag="wosvg")
            nc.scalar.dma_start(out=wosvg, in_=wosv16.rearrange("(h p) c -> p h c", p=64))
            adjtb, whab = [], []
            for g in range(NG):
                t = sb.tile([128, GB, R], f16, tag=f"adjtb{g}", name=f"adjtb{g}")
                nc.sync.dma_start(out=t, in_=adjt_g[g])
                adjtb.append(t)
                t = sb.tile([128, GB, H * HC], f16, tag=f"whab{g}", name=f"whab{g}")
                nc.sync.dma_start(out=t, in_=wha_g[g])
                whab.append(t)
            adjts = [adjtb[jb // GB][:, jb % GB, :] for jb in range(JB)]
            whas = [whab[jb // GB][:, jb % GB, :] for jb in range(JB)]
            evs = [evb[:, jb, :] for jb in range(JB)]
            f1bs = {h: f1bt[:, h, :] for h in range(H)}
            s1rbs = {h: s1rt[:, h, :] for h in range(H)}

            hcTs = [
                sb.tile([64, R], f16, tag=f"hcT{h}", name=f"hcT{h}")
                for h in range(H)
            ]
            rs16v = sb.tile([65, R], f16, tag="rs16v")
            rcp16 = sb.tile([65, R], f16, tag="rcp16")

            with (
                tc.tile_pool(name="ap", bufs=1, space="PSUM") as accp,
                tc.tile_pool(name="fz", bufs=1, space="PSUM") as fzp,
            ):
                outts = {}
                counts = {}
                pending = {"dve": [], "act": [], "pool": []}
                PEND_DEPTH = {"dve": 1, "act": 1, "pool": 1}

                # static psum bank plan: heads sharing a tag have disjoint
                # accumulation lifetimes, so no ring wait can convoy PE
                BANK_TAG = {
                    (0, "sc"): "tA", (3, "sc"): "tA",
                    (1, "sc"): "tB", (2, "sc"): "tB",
                    (7, "sc"): "tC", (6, "sc"): "tC",
                    (5, "act"): "tD", (4, "sc"): "tD",
                    (6, "act"): "tE",
                    (5, "sc"): "tD", (4, "act"): "tE", (7, "act"): "tE",
                }

                def get_outt(h, cls):
                    key = (h, cls)
                    if key not in outts:
                        outts[key] = accp.tile(
                            [HC, R], f32, tag=BANK_TAG[key],
                            name=f"outt_{cls}{h}", bufs=1,
                        )
                        counts[key] = 0
                    return outts[key]

                def emit_vmm(h, cls, jb, p):
                    outt = get_outt(h, cls)
                    counts[(h, cls)] += 1
                    nc.tensor.matmul(
                        outt, whas[jb][:, h * HC:(h + 1) * HC], p,
                        start=(counts[(h, cls)] == 1),
                        stop=(counts[(h, cls)] == expected[(h, cls)]),
                    )

                def flush(e, all_=True):
                    while pending[e] and (all_ or len(pending[e]) > PEND_DEPTH[e]):
                        for args in pending[e].pop(0):
                            emit_vmm(*args)

                dve_units_left = [len(streams["dve"])]

                def emit_scaled(ename, h, jbs):
                    # n tensor_scalars (one per block: the per-partition
                    # scalars differ per key block) + ONE wide mask multiply
                    # over the whole aligned run; value matmuls deferred one
                    # unit so PE never waits on an in-flight p.  Late DVE
                    # units hand the tail columns of the mask multiply to
                    # Pool, which has gone idle by then.
                    eng = {"dve": nc.vector, "pool": nc.gpsimd}[ename]
                    n = len(jbs)
                    g, b0 = jbs[0] // GB, jbs[0] % GB
                    w = pw.tile([128, n, R], f16, tag=f"w{n}_{ename}", bufs=2 if n == 4 else 4)
                    for k, jb in enumerate(jbs):
                        eng.tensor_scalar(
                            w[:, k, :], f1bs[h], evs[jb][:, H + h:H + h + 1],
                            evs[jb][:, h:h + 1], op0=OP.mult, op1=OP.max,
                        )
                    p = pw.tile([128, n, R], f16, tag=f"p{n}_{ename}", bufs=3 if n == 4 else 4)
                    split = 0
                    if ename == "dve":
                        dve_units_left[0] -= 1
                        if dve_units_left[0] < 15:
                            split = 320
                    if split:
                        eng.tensor_tensor(
                            p[:, :, 0:split], w[:, :, 0:split],
                            adjtb[g][:, b0:b0 + n, 0:split], op=OP.mult,
                        )
                        nc.gpsimd.tensor_tensor(
                            p[:, :, split:R], w[:, :, split:R],
                            adjtb[g][:, b0:b0 + n, split:R], op=OP.mult,
                        )
                    else:
                        eng.tensor_tensor(
                            p, w, adjtb[g][:, b0:b0 + n, :], op=OP.mult
                        )
                    pending[ename].append(
                        [(h, "sc", jb, p[:, k, :]) for k, jb in enumerate(jbs)]
                    )
                    flush(ename, all_=False)

                def emit_act(h, jbs):
                    (jb,) = jbs
                    eps = fzp.tile([128, R], f32, tag="eps", bufs=2)
                    nc.tensor.matmul(eps, ones16, s1rbs[h], start=True, stop=False)
                    nc.tensor.matmul(eps, bigi, adjts[jb], start=False, stop=True)
                    m = pw.tile([128, R], f16, tag="m", bufs=4)
                    nc.scalar.activation(
                        m, eps, AF.Prelu, alpha=ALPHA,
                        bias=evb[:, jb, 5 * H + h:5 * H + h + 1],
                    )
                    p = pw.tile([128, R], f16, tag="p2", bufs=5)
                    nc.scalar.activation(p, m, AF.Exp)
                    pending["act"].append([(h, "act", jb, p)])
                    flush("act", all_=False)

                fin_pending = []

                def finalize_a(h, fin_eng):
                    # transposed finalize.  HW allows only ONE psum input per
                    # vector op, so the recip broadcast is copied to SBUF
                    # (on the finishing stream's engine) before the multiply.
                    if h in straddle_heads:
                        osc, oac = outts[(h, "sc")], outts[(h, "act")]
                        cmb = pw.tile([65, R], f16, tag="cmb", bufs=1)
                        nc.vector.tensor_tensor(cmb, oac, e1bt, op=OP.mult)
                        nc.vector.tensor_tensor(
                            rs16v[64:65, :], osc[64:65, :], cmb[64:65, :], op=OP.add
                        )
                        tq = pw.tile([64, R], f16, tag="tq", bufs=2)
                        nc.vector.scalar_tensor_tensor(
                            tq, osc[0:64, :], 0.0, cmb[0:64, :],
                            op0=OP.add, op1=OP.add,
                        )
                        num = tq
                        with nc.allow_low_precision(reason="softmax recip row"):
                            nc.vector.reciprocal(rcp16[64:65, :], rs16v[64:65, :])
                    else:
                        osc = outts.get((h, "sc"))
                        if osc is None:
                            osc = outts[(h, "act")]
                        num = osc[0:64, :]
                        with nc.allow_low_precision(reason="softmax recip row"):
                            nc.vector.reciprocal(rcp16[64:65, :], osc[64:65, :])
                    rb = fzp.tile([64, R], f32, tag="rb")
                    nc.tensor.matmul(
                        rb, ones65[64:65, :], rcp16[64:65, :], start=True, stop=True
                    )
                    rb16 = pw.tile([64, R], f16, tag="rb16", bufs=2)
                    nc.vector.tensor_copy(rb16, rb)
                    t = pw.tile([64, R], f16, tag="t", bufs=2)
                    nc.vector.tensor_tensor(t, num, rb16, op=OP.mult)
                    m0 = pw.tile([64, R], f16, tag="m0", bufs=2)
                    nc.vector.tensor_scalar(
                        m0, t, 1.0 / 64.0, 0.0, op0=OP.mult, op1=OP.min
                    )
                    ex = pw.tile([64, R], f16, tag="ex", bufs=3)
                    nc.scalar.activation(ex, m0, AF.Exp)
                    rl2 = pw.tile([64, R], f16, tag="rl2", bufs=3)
                    nc.vector.tensor_scalar(
                        rl2, t, 1.0 / 64.0, 0.0, op0=OP.mult, op1=OP.max
                    )
                    fin_pending.append([0, h, ex, rl2])

                def fin_tick(force=False):
                    for item in list(fin_pending):
                        item[0] += 1
                        if force or item[0] > 2:
                            _, h, ex, rl2 = item
                            nc.vector.tensor_tensor(hcTs[h], ex, rl2, op=OP.add)
                            nc.vector.tensor_scalar(
                                hcTs[h], hcTs[h], -1.0, None, op0=OP.add
                            )
                            fin_pending.remove(item)

                # --- merged emission by virtual engine clocks ---
                # finalize is deferred 2 units behind the stream that emitted
                # the head's last block: the engine executes behind emission,
                # and an early inline finalize stalls DVE's in-order stream
                fin_a_pending = {"dve": [], "act": [], "pool": []}

                FIN_DEPTH = {"dve": 2, "act": 2, "pool": 2}

                def fin_a_tick(e, force=False):
                    for item in list(fin_a_pending[e]):
                        item[0] += 1
                        if force or item[0] > FIN_DEPTH[e]:
                            finalize_a(item[1], e)
                            clocks["dve"] += 2300.0
                            clocks["act"] += 831.0
                            fin_a_pending[e].remove(item)

                clocks = {"dve": 0.0, "act": 0.0, "pool": 0.0}
                pos = {e: 0 for e in streams}
                head_done = {h: 0 for h in range(H)}
                remaining = sum(len(u) for u in streams.values())
                while remaining:
                    cand = []
                    for e in streams:
                        if pos[e] < len(streams[e]):
                            n = len(streams[e][pos[e]][1])
                            cand.append((clocks[e] + _unit_cost(e, n), e))
                    _, e = min(cand)
                    h, jbs = streams[e][pos[e]]
                    pos[e] += 1
                    remaining -= 1
                    clocks[e] += _unit_cost(e, len(jbs))
                    if e == "act":
                        emit_act(h, jbs)
                    else:
                        emit_scaled(e, h, jbs)
                    head_done[h] += len(jbs)
                    fin_tick()
                    fin_a_tick(e)
                    if pos[e] == len(streams[e]):
                        for e2 in ("dve", "act", "pool"):
                            flush(e2)
                        fin_a_tick(e, force=True)
                    if head_done[h] == JB:
                        for e2 in ("dve", "act", "pool"):
                            flush(e2)
                        fin_a_pending[e].append([0, h])
                for e2 in ("dve", "act", "pool"):
                    flush(e2)
                for e in fin_a_pending:
                    fin_a_tick(e, force=True)
                fin_tick(force=True)

            # --- Who = hcat @ [Wo | w1 | w2] for my rows (64-part stationaries)
            with tc.tile_pool(name="fp2", bufs=2, space="PSUM") as fp2:
                for ic in range(IC):
                    wop = fp2.tile([128, F], f32, tag="wop")
                    svp = fp2.tile([128, 2], f32, tag="svp2")
                    for h in range(H):
                        nc.tensor.matmul(
                            wop, hcTs[h][:, ic * 128:(ic + 1) * 128], woag[:, h, :],
                            start=(h == 0), stop=(h == H - 1),
                        )
                    for h in range(H):
                        nc.tensor.matmul(
                            svp, hcTs[h][:, ic * 128:(ic + 1) * 128], wosvg[:, h, :],
                            start=(h == 0), stop=(h == H - 1),
                        )
                    wt = work.tile([128, F + 1], f16, tag="wt")
                    nc.scalar.activation(wt[:, 0:F], wop, AF.Copy)
                    nc.vector.memset(wt[:, F:F + 1], 1.0)
                    nc.sync.dma_start(
                        out=whoa16[ic * 128:(ic + 1) * 128, :], in_=wt
                    )
                    st = work.tile([128, 3], f32, tag="st")
                    nc.vector.tensor_copy(st[:, 0:2], svp)
                    nc.scalar.activation(st[:, 2:3], svp[:, 0:1], AF.Exp, scale=-0.8)
                    nc.sync.dma_start(out=svo[ic * 128:(ic + 1) * 128, :], in_=st)
    nc.compile()
    return nc


# ---------------------------------------------------------------- k2
def _build_k2():
    """Per-core: output-layer attention for this core's R rows, final ELU.

    in:  whoa [N, F+1] f16, svof [N, 2] f32, svomy [R, 2] f32, adjt [N, R] f16
    out: out [R, F] f32
    """
    nc = bacc.Bacc("TRN2", target_bir_lowering=False, debug=False, num_devices=M)
    whoa = nc.dram_tensor("whoa", [N, F + 1], f16, kind="ExternalInput").ap()
    svof = nc.dram_tensor("svof", [N, 2], f32, kind="ExternalInput").ap()
    svomy = nc.dram_tensor("svomy", [R, 2], f32, kind="ExternalInput").ap()
    bias9_h = nc.dram_tensor("bias9", [128, 1], f32, kind="ExternalInput").ap()
    adjt = nc.dram_tensor("adjt", [N, R], f16, kind="ExternalInput").ap()
    out = nc.dram_tensor("out", [R, F], f16, kind="ExternalOutput").ap()

    with tile.TileContext(nc) as tc:
        with (
            tc.tile_pool(name="sb", bufs=1) as sb,
            tc.tile_pool(name="work", bufs=10) as work,
        ):
            # --- prep first (small DMAs ahead of the big resident loads) ---
            ident = sb.tile([128, 128], f32, tag="ident")
            make_identity(nc, ident)
            ones1 = sb.tile([1, 128], f32, tag="ones1")
            nc.vector.memset(ones1, 1.0)

            # blocked s2 [128, 32]: col b = s2o[b*128 + p]
            s2blk = sb.tile([128, JB], f32, tag="s2blk")
            nc.sync.dma_start(
                out=s2blk, in_=svof.rearrange("(b p) c -> p b c", p=128)[:, :, 1]
            )
            e2c = sb.tile([128, JB], f32, tag="e2c")
            f2c = sb.tile([128, JB], f32, tag="f2c")
            f1bo = sb.tile([128, R], f16, tag="f1bo")

            with tc.tile_pool(name="pp", bufs=2, space="PSUM") as pp:
                # stability shift 9 - max(s2o) comes replicated from the host
                # (a trivial max over k1's svo output - layout-grade prep)
                biasb = sb.tile([128, 1], f32, tag="biasb")
                nc.sync.dma_start(out=biasb, in_=bias9_h)
                nc.scalar.activation(e2c, s2blk, AF.Exp, bias=biasb)
                nc.scalar.activation(f2c, s2blk, AF.Exp, bias=biasb, scale=0.2)

                # F1' broadcast tile from my s1o
                s1row = sb.tile([1, R], f32, tag="s1row")
                nc.sync.dma_start(
                    out=s1row, in_=svomy[:, 0:1].rearrange("r one -> one r")
                )
                f1row = work.tile([1, R], f32, tag="f1row")
                nc.scalar.activation(f1row, s1row, AF.Exp, scale=-0.8)
                fbp = pp.tile([128, R], f32, tag="fbp")
                nc.tensor.matmul(fbp, ones1, f1row, start=True, stop=True)
                nc.scalar.activation(f1bo, fbp, AF.Copy)

            # --- resident loads, coalesced grouped 3D-AP DMAs ---
            GB = 8
            NG = JB // GB
            adjt_g = adjt.rearrange("(g b p) r -> g p b r", b=GB, p=128)
            whoa_g = whoa.rearrange("(g b p) c -> g p b c", b=GB, p=128)
            adjtb, whob = [], []
            for g in range(NG):
                t = sb.tile([128, GB, R], f16, tag=f"adjtb{g}", name=f"adjtb{g}")
                nc.sync.dma_start(out=t, in_=adjt_g[g])
                adjtb.append(t)
                t = sb.tile([128, GB, F + 1], f16, tag=f"whob{g}", name=f"whob{g}")
                nc.sync.dma_start(out=t, in_=whoa_g[g])
                whob.append(t)
            adjts = [adjtb[jb // GB][:, jb % GB, :] for jb in range(JB)]
            whos = [whob[jb // GB][:, jb % GB, :] for jb in range(JB)]

            # --- main loop ---
            with tc.tile_pool(name="ap", bufs=1, space="PSUM") as accp:
                accs = [accp.tile([128, F], f32, tag=f"acc{ic}", name=f"acc{ic}") for ic in range(IC)]
                rss = [accp.tile([128, 1], f32, tag=f"rs{ic}", name=f"rs{ic}") for ic in range(IC)]
                SP = 384  # DVE takes [0:SP), GPSIMD [SP:R) - parallel halves
                for jb in range(JB):
                    w = work.tile([128, R], f16, tag="w")
                    p = work.tile([128, R], f16, tag="p")
                    nc.vector.tensor_scalar(
                        w[:, 0:SP], f1bo[:, 0:SP], f2c[:, jb:jb + 1],
                        e2c[:, jb:jb + 1], op0=OP.mult, op1=OP.max,
                    )
                    nc.vector.tensor_tensor(
                        p[:, 0:SP], w[:, 0:SP], adjts[jb][:, 0:SP], op=OP.mult
                    )
                    nc.gpsimd.tensor_scalar(
                        w[:, SP:R], f1bo[:, SP:R], f2c[:, jb:jb + 1],
                        e2c[:, jb:jb + 1], op0=OP.mult, op1=OP.max,
                    )
                    nc.gpsimd.tensor_tensor(
                        p[:, SP:R], w[:, SP:R], adjts[jb][:, SP:R], op=OP.mult
                    )
                    for ic in range(IC):
                        nc.tensor.matmul(
                            accs[ic], p[:, ic * 128:(ic + 1) * 128],
                            whos[jb][:, 0:F],
                            start=(jb == 0), stop=(jb == JB - 1),
                        )
                        nc.tensor.matmul(
                            rss[ic], p[:, ic * 128:(ic + 1) * 128],
                            whos[jb][:, F:F + 1],
                            start=(jb == 0), stop=(jb == JB - 1),
                        )
                for ic in range(IC):
                    r = work.tile([128, 1], f32, tag="r")
                    nc.vector.reciprocal(r, rss[ic])
                    ot = work.tile([128, F], f16, tag="ot")
                    nc.scalar.activation(ot, accs[ic], AF.Copy, scale=r)
                    m0 = work.tile([128, F], f16, tag="m0")
                    nc.vector.tensor_scalar(m0, ot, 0.0, None, op0=OP.min)
                    ex = work.tile([128, F], f16, tag="ex")
                    nc.scalar.activation(ex, m0, AF.Exp)
                    rl2 = work.tile([128, F], f16, tag="rl2")
                    nc.vector.tensor_scalar(rl2, ot, 0.0, None, op0=OP.max)
                    res = work.tile([128, F], f16, tag="res")
                    nc.vector.tensor_tensor(res, ex, rl2, op=OP.add)
                    nc.vector.tensor_scalar(res, res, -1.0, None, op0=OP.add)
                    nc.sync.dma_start(out=out[ic * 128:(ic + 1) * 128, :], in_=res)
    nc.compile()
    return nc


def _get(name):
    if name not in _CACHE:
        _CACHE[name] = {"k0": _build_k0, "k1": _build_k1, "k2": _build_k2}[name]()
    return _CACHE[name]


# ---------------------------------------------------------------- host
def kernel(x, left, adj, Ws, a1, a2, Wo, ao1, ao2):
    x = np.asarray(x, np.float32)
    adj = np.asarray(adj, np.float32)
    Ws = np.asarray(Ws, np.float32)
    a1 = np.asarray(a1, np.float32)
    a2 = np.asarray(a2, np.float32)
    Wo = np.asarray(Wo, np.float32)
    ao1 = np.asarray(ao1, np.float32)
    ao2 = np.asarray(ao2, np.float32)

    # host-side layout prep (no significant FLOPs)
    ws_all = np.ascontiguousarray(Ws.transpose(1, 0, 2).reshape(F, F))
    ws1 = np.einsum("hkf,hf->kh", Ws, a1)   # [F, H]  tiny matvecs
    ws2 = np.einsum("hkf,hf->kh", Ws, a2)
    wsc = np.ascontiguousarray(
        np.concatenate([ws2, 0.2 * ws2, -0.8 * ws1, -ws1, ws1, ws2], axis=1),
        dtype=np.float32,
    )
    wsa16 = ws_all.astype(np.float16)
    woa = np.ascontiguousarray(Wo).astype(np.float16)
    wosv16 = np.ascontiguousarray(
        np.stack([Wo @ ao1, Wo @ ao2], axis=1), dtype=np.float16
    )
    adj16 = adj.astype(np.float16)  # exact: adj is a 0/1 mask
    adjt_c = [
        np.ascontiguousarray(adj16[c * R:(c + 1) * R].T) for c in range(M)
    ]
    xt_c = [np.ascontiguousarray(x[c * R:(c + 1) * R].T) for c in range(M)]

    cores = list(range(M))

    k0 = _get("k0")
    res0 = _run(
        k0,
        [
            {
                "xT": xt_c[c],
                "xT16": xt_c[c].astype(np.float16),
                "wsa16": wsa16,
                "wsc": wsc,
            }
            for c in cores
        ],
        cores,
    )
    wha = np.concatenate([res0.results[c]["wha16"] for c in cores], axis=0)
    evf = np.concatenate([res0.results[c]["ev"] for c in cores], axis=0)

    # broadcast tables for k1, replicated (layout only) from k0's ev output
    streams, act_heads, scaled_heads = _k1_assignment()
    straddle = sorted(act_heads & scaled_heads)
    in1 = []
    for c in cores:
        evmy = evf[c * R:(c + 1) * R]
        f1ball = np.ascontiguousarray(np.broadcast_to(
            evmy[:, 2 * H:3 * H].T[None, :, :], (128, H, R)
        ).astype(np.float16))
        s1rball = np.ascontiguousarray(
            (evmy[:, 4 * H:5 * H].T[None, :, :] - BIG).astype(np.float16)
        )
        if straddle:
            e1ball = np.ascontiguousarray(np.broadcast_to(
                evmy[:, 3 * H + straddle[0]][None, :], (65, R)
            ).astype(np.float16))
        else:
            e1ball = np.zeros((65, R), np.float16)
        in1.append(
            {
                "wha": wha,
                "evf": evf,
                "f1ball": f1ball,
                "s1rball": s1rball,
                "e1ball": e1ball,
                "adjt": adjt_c[c],
                "woa": woa,
                "wosv16": wosv16,
            }
        )
    k1 = _get("k1")
    res1 = _run(k1, in1, cores)
    whoa = np.concatenate([res1.results[c]["whoa16"] for c in cores], axis=0)
    svof = np.concatenate([res1.results[c]["svo"] for c in cores], axis=0)

    svof2 = np.ascontiguousarray(svof[:, 0:2])
    bias9 = np.full((128, 1), 9.0 - float(svof[:, 1].max()), np.float32)
    k2 = _get("k2")
    res2 = _run(
        k2,
        [
            {
                "whoa": whoa,
                "svof": svof2,
                "svomy": svof2[c * R:(c + 1) * R],
                "bias9": bias9,
                "adjt": adjt_c[c],
            }
            for c in cores
        ],
        cores,
    )
    return np.concatenate(
        [res2.results[c]["out"] for c in cores], axis=0
    ).astype(np.float32)
